# revision 46
# baseline (speedup 1.0000x reference)
"""Batched KDE kernel for Trainium2 (8 NeuronCores, SPMD).

Problem: out[b, n] = sum_m exp(-||Xq[b,n] - Xf[b,m]||^2 / bw[b])
  with Silverman bandwidth bw[b] from Xf; b=4, n=m=4096, d=32.

Sharding: data-parallel over batch b (4 batches x 2 shards of query rows
= 8 cores). Each core handles n_shard=2048 query rows against the full
m=4096 fit set of its batch.

Device algorithm (per core), raw Bass with manual semaphores:
  psum[n, m] = 2*dot - nmu2 via TWO bf16 K=128 matmuls per 512-col chunk
  (bf16 streams at 1 col/cycle; f32 values are split into bf16 pieces
  x = x1+x2+x3+O(2^-24); Q = 2*Xq^T, f = Xf^T, s = f32(f^2)):
    mmA: lhsT=[q1; q1; q1; -1]   rhs=[f1; f2; f3; s1]
    mmB: lhsT=[q2; q2; q3; -1]   rhs=[f1; f2; f1; s2]
  ScalarE activation computes exp(psum/bw - nx2/bw) with a fused
  per-partition accumulate (accum_out) -> the sum over m. ACT is the
  bottleneck (~2.05us per 2048-col group); the schedule keeps its exp
  stream dense and starts it early:
    - bias (-nx2/bw) and scale (1/bw) are host-computed, riding in the
      first scalar-queue DMA (f32 bytes bitcast into the bf16 blob)
    - inputs live in ONE dram blob ordered by first-use, split across
      BOTH HWDGE queues (sync + scalar) for ~2x head bandwidth
    - -1 rows are baked into the blob (no memsets ahead of the PE)
    - PE warmup matmuls on garbage SBUF start immediately so the HAM
      clock gate is released before the first real group
    - exp output is written back IN PLACE to the psum region it reads
      (discarded data; ScalarE->PSUM is the faster port)
    - the final acc->res reduction is split in two so only half of it
      sits after the last exp group
  NOTE: engines run in relaxed ordering mode — any same-engine RAW
  (e.g. vector reduce then add on the same column) needs an explicit
  semaphore between producer and consumer.
Host does sharding/layout/packing plus the 4 scalar bandwidth values and
query norms (global quantile needs a sort; both are O(input) prep).
"""

import numpy as np

B, N, M, D = 4, 4096, 4096, 32
NCORES = 8
SHARDS_PER_BATCH = NCORES // B  # 2
NSHARD = N // SHARDS_PER_BATCH  # 2048
NT = NSHARD // 128  # 16 n-tiles per core
MCHUNK = 512  # matmul free-dim chunk (one psum bank)
ACT_FD = 2048  # activation free dim (4 psum banks)
NG = NT * (M // ACT_FD)  # 32 matmul/exp groups

# blob column offsets (bf16 cols, 64-aligned), ordered by first use and
# grouped into contiguous per-queue transfer ranges. q-piece regions hold
# only 96 data rows (the -1 rows are memset on-device); per-tile lhsT
# pairs [la_t | lb_t] are contiguous so each tile is one small transfer.
OFF_Q0 = 0  # la_t0 (128) | lb_t0 (128)
OFF_CA = 256  # ra m[0:512)
OFF_DA = 768  # rb m[0:512)
OFF_M = 1280  # meta: f32 [128,17] bitcast -> 34 bf16 cols (padded to 64)
OFF_CB = 1344  # ra m[512:1024)
OFF_DB = 1856  # rb m[512:1024)
OFF_G = 2368  # ra m[1024:2048)
OFF_H = 3392  # rb m[1024:2048)
OFF_QT = 4416  # tiles 1..15: [la_t | lb_t] x 256 cols each
OFF_I = 8256  # ra m[2048:4096) (2048)
OFF_J = 10304  # rb m[2048:4096) (2048)
BLOB_W = 12352

_cached = {}


def _la_off(t):
    if t == 0:
        return OFF_Q0
    return OFF_QT + (t - 1) * 256


def _lb_off(t):
    if t == 0:
        return OFF_Q0 + 128
    return OFF_QT + (t - 1) * 256 + 128


def _ra_off(c):  # c = m-col / 512, 0..7
    if c == 0:
        return OFF_CA
    if c == 1:
        return OFF_CB
    if c <= 3:
        return OFF_G + (c - 2) * 512
    return OFF_I + (c - 4) * 512


def _rb_off(c):
    if c == 0:
        return OFF_DA
    if c == 1:
        return OFF_DB
    if c <= 3:
        return OFF_H + (c - 2) * 512
    return OFF_J + (c - 4) * 512


def _build_program():
    import concourse.bass as bass
    import concourse.mybir as mybir
    from contextlib import ExitStack

    nc = bass.Bass()
    f32 = mybir.dt.float32
    bf16 = mybir.dt.bfloat16

    blob = nc.declare_dram_parameter("blob", [128, BLOB_W], bf16, isOutput=False)
    res = nc.declare_dram_parameter("res", [128, NT], f32, isOutput=True)

    with ExitStack() as ctx:
        msb = ctx.enter_context(nc.sbuf_tensor([128, BLOB_W], bf16))
        escr0 = ctx.enter_context(nc.sbuf_tensor([128, ACT_FD], bf16))
        escr1 = ctx.enter_context(nc.sbuf_tensor([128, ACT_FD], bf16))
        escr = [escr0, escr1]
        # slot 2t+h per group; slot NG = split-off first half of group 0
        acc = ctx.enter_context(nc.sbuf_tensor([128, NG + 1], f32))
        res_sb = ctx.enter_context(nc.sbuf_tensor([128, NT], f32))
        warmT = ctx.enter_context(nc.sbuf_tensor([128, 1], f32))
        wscr = ctx.enter_context(nc.sbuf_tensor([128, 640], bf16))
        ps0 = ctx.enter_context(nc.psum_tensor("ps0", [128, ACT_FD], f32))
        ps1 = ctx.enter_context(nc.psum_tensor("ps1", [128, ACT_FD], f32))
        ps = [ps0, ps1]

        s_q0 = ctx.enter_context(nc.semaphore("s_q0"))
        s_cd = ctx.enter_context(nc.semaphore("s_cd"))
        s_m = ctx.enter_context(nc.semaphore("s_m"))
        s_g = ctx.enter_context(nc.semaphore("s_g"))
        s_h2 = ctx.enter_context(nc.semaphore("s_h2"))
        s_t1 = ctx.enter_context(nc.semaphore("s_t1"))
        s_t2 = ctx.enter_context(nc.semaphore("s_t2"))
        s_t36 = ctx.enter_context(nc.semaphore("s_t36"))
        s_t715 = ctx.enter_context(nc.semaphore("s_t715"))
        s_i = ctx.enter_context(nc.semaphore("s_i"))
        s_j = ctx.enter_context(nc.semaphore("s_j"))
        s_ms = ctx.enter_context(nc.semaphore("s_ms"))
        s_s15 = ctx.enter_context(nc.semaphore("s_s15"))
        s_pe = ctx.enter_context(nc.semaphore("s_pe"))
        s_act = ctx.enter_context(nc.semaphore("s_act"))
        s_v1 = ctx.enter_context(nc.semaphore("s_v1"))
        s_v2 = ctx.enter_context(nc.semaphore("s_v2"))
        sem_out = ctx.enter_context(nc.semaphore("sem_out"))
        block = ctx.enter_context(nc.Block())

        meta32 = msb[:, OFF_M : OFF_M + 34].bitcast(f32)  # [128, 17]
        # meta32[:, t] = -nx2/bw for tile t; 1/bw is folded into the
        # matmul operands, so activations use scale=1.0 (immediate)

        @block.sync
        def _(sync):
            sync.dma_start(
                msb[0:96, OFF_Q0:OFF_CA], blob[0:96, OFF_Q0:OFF_CA]
            ).then_inc(s_q0, 16)
            sync.dma_start(msb[:, OFF_CA:OFF_M], blob[:, OFF_CA:OFF_M]).then_inc(
                s_cd, 16
            )
            sync.dma_start(
                msb[0:96, OFF_QT : OFF_QT + 256], blob[0:96, OFF_QT : OFF_QT + 256]
            ).then_inc(s_t1, 16)
            sync.dma_start(msb[:, OFF_G:OFF_H], blob[:, OFF_G:OFF_H]).then_inc(
                s_g, 16
            )
            sync.dma_start(
                msb[0:96, OFF_QT + 512 : OFF_QT + 1536],
                blob[0:96, OFF_QT + 512 : OFF_QT + 1536],
            ).then_inc(s_t36, 16)
            sync.dma_start(
                msb[0:96, OFF_QT + 256 : OFF_QT + 512],
                blob[0:96, OFF_QT + 256 : OFF_QT + 512],
            ).then_inc(s_t2, 16)
            sync.dma_start(
                msb[0:96, OFF_QT + 1536 : OFF_I], blob[0:96, OFF_QT + 1536 : OFF_I]
            ).then_inc(s_t715, 16)
            sync.dma_start(msb[:, OFF_I:OFF_J], blob[:, OFF_I:OFF_J]).then_inc(
                s_i, 16
            )
            sync.dma_start(msb[:, OFF_J:BLOB_W], blob[:, OFF_J:BLOB_W]).then_inc(
                s_j, 16
            )
            sync.wait_ge(s_v1, 1)
            sync.dma_start(res[:, 0:8], res_sb[:, 0:8]).then_inc(sem_out, 16)
            sync.wait_ge(s_v2, 1)
            sync.wait_ge(s_s15, 1)
            # no completion wait: the NEFF teardown drains the DMA queue;
            # skipping it lets the block exit overlap the HBM write receipt
            sync.dma_start(res[:, 8:16], res_sb[:, 8:16]).then_inc(sem_out, 16)

        @block.vector
        def _(vector):
            # -1 rows (96:128) of the q-piece regions, off the DMA path
            nc.vector.memset(msb[96:128, OFF_Q0:OFF_CA], -1.0)
            nc.vector.memset(msb[96:128, OFF_QT:OFF_I], -1.0).then_inc(s_ms, 1)
            # split final reduction: tiles 0-7 as soon as their h=1 groups
            # are done, tiles 8-15 after the last group
            vector.wait_ge(s_act, 1 + 23)
            nc.vector.tensor_reduce(
                res_sb[:, 0:8],
                acc[:, 0:16].rearrange("p (t h) -> p t h", h=2),
                axis=mybir.AxisListType.X,
                op=mybir.AluOpType.add,
            ).then_inc(s_v1, 1)
            # tiles 8-14 reduce one group before the end; tile 15 is summed
            # on the scalar engine right after the final accumulator read
            vector.wait_ge(s_act, 1 + 30)
            nc.vector.tensor_reduce(
                res_sb[:, 8:15],
                acc[:, 16:30].rearrange("p (t h) -> p t h", h=2),
                axis=mybir.AxisListType.X,
                op=mybir.AluOpType.add,
            ).then_inc(s_v2, 1)

        @block.scalar
        def _(scalar):
            # second HWDGE queue: scalar-issued DMAs interleave with the
            # sync queue on the shared port; order both queues by first use
            scalar.dma_start(msb[:, OFF_M:OFF_G], blob[:, OFF_M:OFF_G]).then_inc(
                s_m, 16
            )
            scalar.dma_start(msb[:, OFF_H:OFF_QT], blob[:, OFF_H:OFF_QT]).then_inc(
                s_h2, 16
            )
            # fire the exp table-set load; operands are garbage (meta not
            # yet DMA'd) but the output is discarded
            nc.scalar.activation(
                warmT[:],
                warmT[:],
                mybir.ActivationFunctionType.Exp,
                bias=meta32[:, 0:1],
            )
            scalar.wait_ge(s_m, 16)
            # uniform 2048-col groups; s_pe = g+1 when group g's psum is
            # filled, s_act = g+1 when its exp+accumulate is done
            for g in range(NG):
                t = g % NT
                slot = 2 * t + (g // NT)
                scalar.wait_ge(s_pe, g + 1)
                nc.scalar.activation(
                    escr[g % 2][:],
                    ps[g % 2][:],
                    mybir.ActivationFunctionType.Exp,
                    bias=meta32[:, t : t + 1],
                    accum_out=acc[:, slot : slot + 1],
                ).then_inc(s_act, 1)
            # tile 15 = acc[30] + acc[31]; the self-wait orders this read
            # of acc[31] after the final accumulator read (relaxed ordering)
            scalar.wait_ge(s_act, NG)
            nc.scalar.activation(
                res_sb[:, 15:16],
                acc[:, 30:31],
                mybir.ActivationFunctionType.Identity,
                bias=acc[:, 31:32],
            ).then_inc(s_s15, 1)

        @block.tensor
        def _(tensor):
            # warm the PE clock (HAM) with dummy matmuls on garbage SBUF so
            # group 0 runs at 2.4 GHz; ps0 is overwritten by group 0
            for _w in range(10):
                nc.tensor.matmul(
                    ps0[:, 0:MCHUNK],
                    wscr[:, 0:128],
                    wscr[:, 128:640],
                    start=True,
                    stop=True,
                )
            for g in range(NG):
                t = g % NT
                h = g // NT
                pg = ps[g % 2]
                la = msb[:, _la_off(t) : _la_off(t) + 128]
                lb = msb[:, _lb_off(t) : _lb_off(t) + 128]
                if g == 0:
                    # chunk 1's data (scalar queue) usually lands before
                    # chunk 0's (sync queue) — run the c1 pair first
                    tensor.wait_ge(s_ms, 1)
                    tensor.wait_ge(s_q0, 16)
                    tensor.wait_ge(s_m, 16)
                    nc.tensor.matmul(
                        pg[:, MCHUNK : 2 * MCHUNK],
                        la,
                        msb[:, _ra_off(1) : _ra_off(1) + MCHUNK],
                        start=True,
                        stop=False,
                    )
                    nc.tensor.matmul(
                        pg[:, MCHUNK : 2 * MCHUNK],
                        lb,
                        msb[:, _rb_off(1) : _rb_off(1) + MCHUNK],
                        start=False,
                        stop=True,
                    )
                    tensor.wait_ge(s_cd, 16)
                    nc.tensor.matmul(
                        pg[:, 0:MCHUNK],
                        la,
                        msb[:, _ra_off(0) : _ra_off(0) + MCHUNK],
                        start=True,
                        stop=False,
                    )
                    nc.tensor.matmul(
                        pg[:, 0:MCHUNK],
                        lb,
                        msb[:, _rb_off(0) : _rb_off(0) + MCHUNK],
                        start=False,
                        stop=True,
                    )
                    tensor.wait_ge(s_g, 16)
                    for c in (2, 3):
                        nc.tensor.matmul(
                            pg[:, c * MCHUNK : (c + 1) * MCHUNK],
                            la,
                            msb[:, _ra_off(c) : _ra_off(c) + MCHUNK],
                            start=True,
                            stop=False,
                        )
                    tensor.wait_ge(s_h2, 16)
                    for c in (2, 3):
                        mm = nc.tensor.matmul(
                            pg[:, c * MCHUNK : (c + 1) * MCHUNK],
                            lb,
                            msb[:, _rb_off(c) : _rb_off(c) + MCHUNK],
                            start=False,
                            stop=True,
                        )
                        if c == 3:
                            mm.then_inc(s_pe, 1)
                    continue
                if g == 1:
                    tensor.wait_ge(s_t1, 16)
                if g == 2:
                    tensor.wait_ge(s_t2, 16)
                if g == 3:
                    tensor.wait_ge(s_t36, 16)
                if g == 7:
                    tensor.wait_ge(s_t715, 16)
                if g == 16:
                    tensor.wait_ge(s_i, 16)
                if g >= 2:
                    tensor.wait_ge(s_act, g - 1)
                for j in range(4):
                    c = 4 * h + j
                    nc.tensor.matmul(
                        pg[:, j * MCHUNK : (j + 1) * MCHUNK],
                        la,
                        msb[:, _ra_off(c) : _ra_off(c) + MCHUNK],
                        start=True,
                        stop=False,
                    )
                if g == 16:
                    tensor.wait_ge(s_j, 16)
                for j in range(4):
                    c = 4 * h + j
                    mm = nc.tensor.matmul(
                        pg[:, j * MCHUNK : (j + 1) * MCHUNK],
                        lb,
                        msb[:, _rb_off(c) : _rb_off(c) + MCHUNK],
                        start=False,
                        stop=True,
                    )
                    if j == 3:
                        mm.then_inc(s_pe, 1)

    return nc


def _bf16_split3(x):
    import ml_dtypes

    bf = ml_dtypes.bfloat16
    x = x.astype(np.float32)
    p1 = x.astype(bf)
    rem = x - p1.astype(np.float32)
    p2 = rem.astype(bf)
    rem2 = rem - p2.astype(np.float32)
    p3 = rem2.astype(bf)
    return p1, p2, p3


def _bandwidth_np(X_fit):
    # mirror of reference._bandwidth (Silverman-style)
    b, n, d = X_fit.shape
    flat = np.asarray(X_fit, dtype=np.float64).reshape(-1)
    q = np.quantile(flat, 0.75) - np.quantile(flat, 0.25)
    std = np.std(np.asarray(X_fit, dtype=np.float64).reshape(b, -1), axis=1, ddof=1)
    return (0.9 * np.minimum(std, q / 1.34) / (n**0.2)).astype(np.float32)


def _host_prep(X_query, X_fit):
    import ml_dtypes

    bf = ml_dtypes.bfloat16
    X_query = np.asarray(X_query, dtype=np.float32)
    X_fit = np.asarray(X_fit, dtype=np.float32)
    bw = _bandwidth_np(X_fit)  # [B]

    in_maps = []
    for c in range(NCORES):
        b = c // SHARDS_PER_BATCH
        s = c % SHARDS_PER_BATCH
        XQ = X_query[b, s * NSHARD : (s + 1) * NSHARD]  # [2048, 32]
        XF = X_fit[b]  # [4096, 32]
        inv_bw = np.float32(1.0) / bw[b]

        # permuted queries: tile t / partition p handles query row p*NT + t.
        # 1/bw is folded into the operands so the activation runs with an
        # immediate scale of 1.0 (no per-instruction scale-AP fetch).
        XQp = XQ.reshape(128, NT, D).transpose(1, 0, 2).reshape(NSHARD, D)
        Q = np.ascontiguousarray(
            (2.0 * np.float64(inv_bw) * XQp.T.astype(np.float64)).astype(np.float32)
        )  # [32, 2048]
        q1, q2, q3 = _bf16_split3(Q)
        FT = np.ascontiguousarray(XF.T.astype(np.float32))  # [32, 4096]
        f1, f2, f3 = _bf16_split3(FT)
        sqr = (
            FT.astype(np.float64) ** 2 * np.float64(inv_bw)
        ).astype(np.float32)  # |f|^2 / bw
        s1, s2, _s3 = _bf16_split3(sqr)

        la = np.concatenate([q1, q1, q1], axis=0)  # [96, 2048]
        lb = np.concatenate([q2, q2, q3], axis=0)
        ra = np.concatenate([f1, f2, f3, s1], axis=0)  # [128, 4096]
        rb = np.concatenate([f1, f2, f1, s2], axis=0)

        nx2 = (XQ.reshape(128, NT, D).astype(np.float64) ** 2).sum(-1)
        meta = np.empty((128, 17), dtype=np.float32)
        meta[:, 0:16] = (-nx2 * np.float64(inv_bw)).astype(np.float32)
        meta[:, 16] = inv_bw

        blob = np.zeros((128, BLOB_W), dtype=bf)
        for t in range(NT):
            lo = OFF_Q0 if t == 0 else OFF_QT + (t - 1) * 256
            blob[0:96, lo : lo + 128] = la[:, t * 128 : (t + 1) * 128]
            blob[0:96, lo + 128 : lo + 256] = lb[:, t * 128 : (t + 1) * 128]
        blob[:, OFF_CA : OFF_CA + 512] = ra[:, 0:512]
        blob[:, OFF_DA : OFF_DA + 512] = rb[:, 0:512]
        blob[:, OFF_M : OFF_M + 34] = meta.view(np.uint16).view(bf)  # raw bytes
        blob[:, OFF_CB : OFF_CB + 512] = ra[:, 512:1024]
        blob[:, OFF_DB : OFF_DB + 512] = rb[:, 512:1024]
        blob[:, OFF_G : OFF_G + 1024] = ra[:, 1024:2048]
        blob[:, OFF_H : OFF_H + 1024] = rb[:, 1024:2048]
        blob[:, OFF_I : OFF_I + 2048] = ra[:, 2048:4096]
        blob[:, OFF_J : OFF_J + 2048] = rb[:, 2048:4096]

        in_maps.append({"blob": blob})
    return in_maps


def _gather(results):
    out = np.empty((B, N), dtype=np.float32)
    for c in range(NCORES):
        b = c // SHARDS_PER_BATCH
        s = c % SHARDS_PER_BATCH
        res = np.asarray(results[c]["res"], dtype=np.float32)  # [128, 16]
        out[b, s * NSHARD : (s + 1) * NSHARD] = res.reshape(NSHARD)
    return out


def kernel(X_query, X_fit):
    from concourse.bass_utils import run_bass_kernel_spmd

    if "nc" not in _cached:
        _cached["nc"] = _build_program()
    nc = _cached["nc"]
    in_maps = _host_prep(X_query, X_fit)
    out = run_bass_kernel_spmd(nc, in_maps, list(range(NCORES)))
    return _gather(out.results)


# revision 50
# speedup vs baseline: 1.0254x; 1.0254x over previous
"""Batched KDE kernel for Trainium2 (8 NeuronCores, SPMD).

Problem: out[b, n] = sum_m exp(-||Xq[b,n] - Xf[b,m]||^2 / bw[b])
  with Silverman bandwidth bw[b] from Xf; b=4, n=m=4096, d=32.

Sharding: data-parallel over batch b (4 batches x 2 shards of query rows
= 8 cores). Each core handles n_shard=2048 query rows against the full
m=4096 fit set of its batch.

Device algorithm (per core), raw Bass with manual semaphores:
  psum[n, m] = 2*dot - nmu2 via TWO bf16 K=128 matmuls per 512-col chunk
  (bf16 streams at 1 col/cycle; f32 values are split into bf16 pieces
  x = x1+x2+x3+O(2^-24); Q = 2*Xq^T, f = Xf^T, s = f32(f^2)):
    mmA: lhsT=[q1; q1; q1; -1]   rhs=[f1; f2; f3; s1]
    mmB: lhsT=[q2; q2; q3; -1]   rhs=[f1; f2; f1; s2]
  ScalarE activation computes exp(psum/bw - nx2/bw) with a fused
  per-partition accumulate (accum_out) -> the sum over m. ACT is the
  bottleneck (~2.05us per 2048-col group); the schedule keeps its exp
  stream dense and starts it early:
    - bias (-nx2/bw) and scale (1/bw) are host-computed, riding in the
      first scalar-queue DMA (f32 bytes bitcast into the bf16 blob)
    - inputs live in ONE dram blob ordered by first-use, split across
      BOTH HWDGE queues (sync + scalar) for ~2x head bandwidth
    - -1 rows are baked into the blob (no memsets ahead of the PE)
    - PE warmup matmuls on garbage SBUF start immediately so the HAM
      clock gate is released before the first real group
    - exp output is written back IN PLACE to the psum region it reads
      (discarded data; ScalarE->PSUM is the faster port)
    - the final acc->res reduction is split in two so only half of it
      sits after the last exp group
  NOTE: engines run in relaxed ordering mode — any same-engine RAW
  (e.g. vector reduce then add on the same column) needs an explicit
  semaphore between producer and consumer.
Host does sharding/layout/packing plus the 4 scalar bandwidth values and
query norms (global quantile needs a sort; both are O(input) prep).
"""

import numpy as np

B, N, M, D = 4, 4096, 4096, 32
NCORES = 8
SHARDS_PER_BATCH = NCORES // B  # 2
NSHARD = N // SHARDS_PER_BATCH  # 2048
NT = NSHARD // 128  # 16 n-tiles per core
MCHUNK = 512  # matmul free-dim chunk (one psum bank)
ACT_FD = 2048  # activation free dim (4 psum banks)
NG = NT * (M // ACT_FD)  # 32 matmul/exp groups

# blob column offsets (bf16 cols, 64-aligned), ordered by first use and
# grouped into contiguous per-queue transfer ranges. q-piece regions hold
# only 96 data rows (the -1 rows are memset on-device); per-tile lhsT
# pairs [la_t | lb_t] are contiguous so each tile is one small transfer.
OFF_Q0 = 0  # la_t0 (128) | lb_t0 (128)
OFF_CA = 256  # ra m[0:512)
OFF_DA = 768  # rb m[0:512)
OFF_M = 1280  # meta: f32 [128,17] bitcast -> 34 bf16 cols (padded to 64)
OFF_CB = 1344  # ra m[512:1024)
OFF_DB = 1856  # rb m[512:1024)
OFF_G = 2368  # ra m[1024:2048)
OFF_H = 3392  # rb m[1024:2048)
OFF_QT = 4416  # tiles 1..15: [la_t | lb_t] x 256 cols each
OFF_I = 8256  # ra m[2048:4096) (2048)
OFF_J = 10304  # rb m[2048:4096) (2048)
BLOB_W = 12352

_cached = {}


def _la_off(t):
    if t == 0:
        return OFF_Q0
    return OFF_QT + (t - 1) * 256


def _lb_off(t):
    if t == 0:
        return OFF_Q0 + 128
    return OFF_QT + (t - 1) * 256 + 128


def _ra_off(c):  # c = m-col / 512, 0..7
    if c == 0:
        return OFF_CA
    if c == 1:
        return OFF_CB
    if c <= 3:
        return OFF_G + (c - 2) * 512
    return OFF_I + (c - 4) * 512


def _rb_off(c):
    if c == 0:
        return OFF_DA
    if c == 1:
        return OFF_DB
    if c <= 3:
        return OFF_H + (c - 2) * 512
    return OFF_J + (c - 4) * 512


def _build_program():
    import concourse.bass as bass
    import concourse.mybir as mybir
    from contextlib import ExitStack

    nc = bass.Bass()
    f32 = mybir.dt.float32
    bf16 = mybir.dt.bfloat16

    blob = nc.declare_dram_parameter("blob", [128, BLOB_W], bf16, isOutput=False)
    res = nc.declare_dram_parameter("res", [128, NT], f32, isOutput=True)

    with ExitStack() as ctx:
        msb = ctx.enter_context(nc.sbuf_tensor([128, BLOB_W], bf16))
        escr0 = ctx.enter_context(nc.sbuf_tensor([128, ACT_FD], bf16))
        escr1 = ctx.enter_context(nc.sbuf_tensor([128, ACT_FD], bf16))
        escr = [escr0, escr1]
        # slot 2t+h per group; slot NG = split-off first half of group 0
        acc = ctx.enter_context(nc.sbuf_tensor([128, NG + 1], f32))
        res_sb = ctx.enter_context(nc.sbuf_tensor([128, NT], f32))
        warmT = ctx.enter_context(nc.sbuf_tensor([128, 1], f32))
        wscr = ctx.enter_context(nc.sbuf_tensor([128, 640], bf16))
        ps0 = ctx.enter_context(nc.psum_tensor("ps0", [128, ACT_FD], f32))
        ps1 = ctx.enter_context(nc.psum_tensor("ps1", [128, ACT_FD], f32))
        ps = [ps0, ps1]

        s_q0 = ctx.enter_context(nc.semaphore("s_q0"))
        s_cd = ctx.enter_context(nc.semaphore("s_cd"))
        s_m = ctx.enter_context(nc.semaphore("s_m"))
        s_g = ctx.enter_context(nc.semaphore("s_g"))
        s_h2 = ctx.enter_context(nc.semaphore("s_h2"))
        s_t1 = ctx.enter_context(nc.semaphore("s_t1"))
        s_t2 = ctx.enter_context(nc.semaphore("s_t2"))
        s_t36 = ctx.enter_context(nc.semaphore("s_t36"))
        s_t715 = ctx.enter_context(nc.semaphore("s_t715"))
        s_i = ctx.enter_context(nc.semaphore("s_i"))
        s_j = ctx.enter_context(nc.semaphore("s_j"))
        s_ms = ctx.enter_context(nc.semaphore("s_ms"))
        s_pe = ctx.enter_context(nc.semaphore("s_pe"))
        s_act = ctx.enter_context(nc.semaphore("s_act"))
        s_v1 = ctx.enter_context(nc.semaphore("s_v1"))
        s_v2 = ctx.enter_context(nc.semaphore("s_v2"))
        sem_out = ctx.enter_context(nc.semaphore("sem_out"))
        block = ctx.enter_context(nc.Block())

        meta32 = msb[:, OFF_M : OFF_M + 34].bitcast(f32)  # [128, 17]
        # meta32[:, t] = -nx2/bw for tile t; 1/bw is folded into the
        # matmul operands, so activations use scale=1.0 (immediate)

        @block.sync
        def _(sync):
            sync.dma_start(
                msb[0:96, OFF_Q0:OFF_CA], blob[0:96, OFF_Q0:OFF_CA]
            ).then_inc(s_q0, 16)
            sync.dma_start(msb[:, OFF_CA:OFF_M], blob[:, OFF_CA:OFF_M]).then_inc(
                s_cd, 16
            )
            sync.dma_start(
                msb[0:96, OFF_QT : OFF_QT + 256], blob[0:96, OFF_QT : OFF_QT + 256]
            ).then_inc(s_t1, 16)
            sync.dma_start(msb[:, OFF_G:OFF_H], blob[:, OFF_G:OFF_H]).then_inc(
                s_g, 16
            )
            sync.dma_start(
                msb[0:96, OFF_QT + 512 : OFF_QT + 1536],
                blob[0:96, OFF_QT + 512 : OFF_QT + 1536],
            ).then_inc(s_t36, 16)
            sync.dma_start(
                msb[0:96, OFF_QT + 256 : OFF_QT + 512],
                blob[0:96, OFF_QT + 256 : OFF_QT + 512],
            ).then_inc(s_t2, 16)
            sync.dma_start(
                msb[0:96, OFF_QT + 1536 : OFF_I], blob[0:96, OFF_QT + 1536 : OFF_I]
            ).then_inc(s_t715, 16)
            sync.dma_start(msb[:, OFF_I:OFF_J], blob[:, OFF_I:OFF_J]).then_inc(
                s_i, 16
            )
            sync.dma_start(msb[:, OFF_J:BLOB_W], blob[:, OFF_J:BLOB_W]).then_inc(
                s_j, 16
            )
            sync.wait_ge(s_v1, 1)
            sync.dma_start(res[:, 0:8], res_sb[:, 0:8]).then_inc(sem_out, 16)
            sync.wait_ge(s_v2, 1)
            # no completion wait: the NEFF teardown drains the DMA queue;
            # skipping it lets the block exit overlap the HBM write receipt
            sync.dma_start(res[:, 8:16], res_sb[:, 8:16]).then_inc(sem_out, 16)

        @block.vector
        def _(vector):
            # -1 rows (96:128) of the q-piece regions, off the DMA path
            nc.vector.memset(msb[96:128, OFF_Q0:OFF_CA], -1.0)
            nc.vector.memset(msb[96:128, OFF_QT:OFF_I], -1.0).then_inc(s_ms, 1)
            # split final reduction: tiles 0-7 as soon as their h=1 groups
            # are done, tiles 8-15 after the last group
            vector.wait_ge(s_act, 1 + 23)
            nc.vector.tensor_reduce(
                res_sb[:, 0:8],
                acc[:, 0:16].rearrange("p (t h) -> p t h", h=2),
                axis=mybir.AxisListType.X,
                op=mybir.AluOpType.add,
            ).then_inc(s_v1, 1)
            vector.wait_ge(s_act, 1 + 31)
            nc.vector.tensor_reduce(
                res_sb[:, 8:16],
                acc[:, 16:32].rearrange("p (t h) -> p t h", h=2),
                axis=mybir.AxisListType.X,
                op=mybir.AluOpType.add,
            ).then_inc(s_v2, 1)

        @block.scalar
        def _(scalar):
            # second HWDGE queue: scalar-issued DMAs interleave with the
            # sync queue on the shared port; order both queues by first use
            scalar.dma_start(msb[:, OFF_M:OFF_G], blob[:, OFF_M:OFF_G]).then_inc(
                s_m, 16
            )
            scalar.dma_start(msb[:, OFF_H:OFF_QT], blob[:, OFF_H:OFF_QT]).then_inc(
                s_h2, 16
            )
            # fire the exp table-set load; operands are garbage (meta not
            # yet DMA'd) but the output is discarded
            nc.scalar.activation(
                warmT[:],
                warmT[:],
                mybir.ActivationFunctionType.Exp,
                bias=meta32[:, 0:1],
            )
            scalar.wait_ge(s_m, 16)
            # uniform 2048-col groups; s_pe = g+1 when group g's psum is
            # filled, s_act = g+1 when its exp+accumulate is done
            for g in range(NG):
                t = g % NT
                slot = 2 * t + (g // NT)
                scalar.wait_ge(s_pe, g + 1)
                # direct InstActivation: bias/scale as immediates (the
                # bias is folded into mmB, the scale into the operands) —
                # saves two per-instruction operand-AP fetches (~180ns)
                nc.scalar.add_instruction(
                    mybir.InstActivation(
                        name=nc.get_next_instruction_name(),
                        func=mybir.ActivationFunctionType.Exp,
                        ins=[
                            nc.scalar.lower_ap(ps[g % 2][:]),
                            mybir.ImmediateValue(
                                dtype=mybir.dt.float32, value=0.0
                            ),
                            mybir.ImmediateValue(
                                dtype=mybir.dt.float32, value=1.0
                            ),
                            mybir.ImmediateValue(
                                dtype=mybir.dt.float32, value=0.0
                            ),
                        ],
                        outs=[
                            nc.scalar.lower_ap(escr[g % 2][:]),
                            nc.scalar.lower_ap(acc[:, slot : slot + 1]),
                        ],
                    )
                ).then_inc(s_act, 1)

        @block.tensor
        def _(tensor):
            # warm the PE clock (HAM) with dummy matmuls on garbage SBUF so
            # group 0 runs at 2.4 GHz; ps0 is overwritten by group 0
            for _w in range(10):
                nc.tensor.matmul(
                    ps0[:, 0:MCHUNK],
                    wscr[:, 0:128],
                    wscr[:, 128:640],
                    start=True,
                    stop=True,
                )
            for g in range(NG):
                t = g % NT
                h = g // NT
                pg = ps[g % 2]
                la = msb[:, _la_off(t) : _la_off(t) + 128]
                lb = msb[:, _lb_off(t) : _lb_off(t) + 128]
                if g == 0:
                    # chunk 1's data (scalar queue) usually lands before
                    # chunk 0's (sync queue) — run the c1 pair first
                    tensor.wait_ge(s_ms, 1)
                    tensor.wait_ge(s_q0, 16)
                    tensor.wait_ge(s_m, 16)
                    nc.tensor.matmul(
                        pg[:, MCHUNK : 2 * MCHUNK],
                        la,
                        msb[:, _ra_off(1) : _ra_off(1) + MCHUNK],
                        start=True,
                        stop=False,
                    )
                    nc.tensor.matmul(
                        pg[:, MCHUNK : 2 * MCHUNK],
                        lb,
                        msb[:, _rb_off(1) : _rb_off(1) + MCHUNK],
                        start=False,
                        stop=True,
                    )
                    tensor.wait_ge(s_cd, 16)
                    nc.tensor.matmul(
                        pg[:, 0:MCHUNK],
                        la,
                        msb[:, _ra_off(0) : _ra_off(0) + MCHUNK],
                        start=True,
                        stop=False,
                    )
                    nc.tensor.matmul(
                        pg[:, 0:MCHUNK],
                        lb,
                        msb[:, _rb_off(0) : _rb_off(0) + MCHUNK],
                        start=False,
                        stop=True,
                    )
                    tensor.wait_ge(s_g, 16)
                    for c in (2, 3):
                        nc.tensor.matmul(
                            pg[:, c * MCHUNK : (c + 1) * MCHUNK],
                            la,
                            msb[:, _ra_off(c) : _ra_off(c) + MCHUNK],
                            start=True,
                            stop=False,
                        )
                    tensor.wait_ge(s_h2, 16)
                    for c in (2, 3):
                        mm = nc.tensor.matmul(
                            pg[:, c * MCHUNK : (c + 1) * MCHUNK],
                            lb,
                            msb[:, _rb_off(c) : _rb_off(c) + MCHUNK],
                            start=False,
                            stop=True,
                        )
                        if c == 3:
                            mm.then_inc(s_pe, 1)
                    continue
                if g == 1:
                    tensor.wait_ge(s_t1, 16)
                if g == 2:
                    tensor.wait_ge(s_t2, 16)
                if g == 3:
                    tensor.wait_ge(s_t36, 16)
                if g == 7:
                    tensor.wait_ge(s_t715, 16)
                if g == 16:
                    tensor.wait_ge(s_i, 16)
                if g >= 2:
                    tensor.wait_ge(s_act, g - 1)
                for j in range(4):
                    c = 4 * h + j
                    nc.tensor.matmul(
                        pg[:, j * MCHUNK : (j + 1) * MCHUNK],
                        la,
                        msb[:, _ra_off(c) : _ra_off(c) + MCHUNK],
                        start=True,
                        stop=False,
                    )
                if g == 16:
                    tensor.wait_ge(s_j, 16)
                for j in range(4):
                    c = 4 * h + j
                    mm = nc.tensor.matmul(
                        pg[:, j * MCHUNK : (j + 1) * MCHUNK],
                        lb,
                        msb[:, _rb_off(c) : _rb_off(c) + MCHUNK],
                        start=False,
                        stop=True,
                    )
                    if j == 3:
                        mm.then_inc(s_pe, 1)

    return nc


def _bf16_split3(x):
    import ml_dtypes

    bf = ml_dtypes.bfloat16
    x = x.astype(np.float32)
    p1 = x.astype(bf)
    rem = x - p1.astype(np.float32)
    p2 = rem.astype(bf)
    rem2 = rem - p2.astype(np.float32)
    p3 = rem2.astype(bf)
    return p1, p2, p3


def _bandwidth_np(X_fit):
    # mirror of reference._bandwidth (Silverman-style)
    b, n, d = X_fit.shape
    flat = np.asarray(X_fit, dtype=np.float64).reshape(-1)
    q = np.quantile(flat, 0.75) - np.quantile(flat, 0.25)
    std = np.std(np.asarray(X_fit, dtype=np.float64).reshape(b, -1), axis=1, ddof=1)
    return (0.9 * np.minimum(std, q / 1.34) / (n**0.2)).astype(np.float32)


def _host_prep(X_query, X_fit):
    import ml_dtypes

    bf = ml_dtypes.bfloat16
    X_query = np.asarray(X_query, dtype=np.float32)
    X_fit = np.asarray(X_fit, dtype=np.float32)
    bw = _bandwidth_np(X_fit)  # [B]

    in_maps = []
    for c in range(NCORES):
        b = c // SHARDS_PER_BATCH
        s = c % SHARDS_PER_BATCH
        XQ = X_query[b, s * NSHARD : (s + 1) * NSHARD]  # [2048, 32]
        XF = X_fit[b]  # [4096, 32]
        inv_bw = np.float32(1.0) / bw[b]

        # permuted queries: tile t / partition p handles query row p*NT + t.
        # 1/bw is folded into the operands so the activation runs with an
        # immediate scale of 1.0 (no per-instruction scale-AP fetch).
        XQp = XQ.reshape(128, NT, D).transpose(1, 0, 2).reshape(NSHARD, D)
        Q = np.ascontiguousarray(
            (2.0 * np.float64(inv_bw) * XQp.T.astype(np.float64)).astype(np.float32)
        )  # [32, 2048]
        q1, q2, q3 = _bf16_split3(Q)
        FT = np.ascontiguousarray(XF.T.astype(np.float32))  # [32, 4096]
        f1, f2, f3 = _bf16_split3(FT)
        sqr = (
            FT.astype(np.float64) ** 2 * np.float64(inv_bw)
        ).astype(np.float32)  # |f|^2 / bw
        s1, s2, _s3 = _bf16_split3(sqr)

        # the per-row bias (-|q|^2/bw) rides inside mmB as two bf16 lhsT
        # rows against rhs rows of ones, so the activation needs neither a
        # bias nor a scale operand (both immediates); q3 keeps 30 of 32
        # dims to make room (the dropped q3*f1 tail is ~5e-4 on the arg)
        nx2 = (XQ.reshape(128, NT, D).astype(np.float64) ** 2).sum(-1)
        br = (
            ((-nx2) * np.float64(inv_bw)).astype(np.float32).T.reshape(1, NSHARD)
        )  # col t*128+a = bias for tile t, partition a
        b1 = br.astype(bf)
        b2 = (br - b1.astype(np.float32)).astype(bf)
        ones2 = np.ones((2, M), dtype=bf)

        la = np.concatenate([q1, q1, q1], axis=0)  # [96, 2048]
        lb = np.concatenate([q2, q2, b1, b2, q3[:30]], axis=0)
        ra = np.concatenate([f1, f2, f3, s1], axis=0)  # [128, 4096]
        rb = np.concatenate([f1, f2, ones2, f1[:30], s2], axis=0)

        meta = np.empty((128, 17), dtype=np.float32)
        meta[:, 0:16] = (-nx2 * np.float64(inv_bw)).astype(np.float32)
        meta[:, 16] = inv_bw

        blob = np.zeros((128, BLOB_W), dtype=bf)
        for t in range(NT):
            lo = OFF_Q0 if t == 0 else OFF_QT + (t - 1) * 256
            blob[0:96, lo : lo + 128] = la[:, t * 128 : (t + 1) * 128]
            blob[0:96, lo + 128 : lo + 256] = lb[:, t * 128 : (t + 1) * 128]
        blob[:, OFF_CA : OFF_CA + 512] = ra[:, 0:512]
        blob[:, OFF_DA : OFF_DA + 512] = rb[:, 0:512]
        blob[:, OFF_M : OFF_M + 34] = meta.view(np.uint16).view(bf)  # raw bytes
        blob[:, OFF_CB : OFF_CB + 512] = ra[:, 512:1024]
        blob[:, OFF_DB : OFF_DB + 512] = rb[:, 512:1024]
        blob[:, OFF_G : OFF_G + 1024] = ra[:, 1024:2048]
        blob[:, OFF_H : OFF_H + 1024] = rb[:, 1024:2048]
        blob[:, OFF_I : OFF_I + 2048] = ra[:, 2048:4096]
        blob[:, OFF_J : OFF_J + 2048] = rb[:, 2048:4096]

        in_maps.append({"blob": blob})
    return in_maps


def _gather(results):
    out = np.empty((B, N), dtype=np.float32)
    for c in range(NCORES):
        b = c // SHARDS_PER_BATCH
        s = c % SHARDS_PER_BATCH
        res = np.asarray(results[c]["res"], dtype=np.float32)  # [128, 16]
        out[b, s * NSHARD : (s + 1) * NSHARD] = res.reshape(NSHARD)
    return out


def kernel(X_query, X_fit):
    from concourse.bass_utils import run_bass_kernel_spmd

    if "nc" not in _cached:
        _cached["nc"] = _build_program()
    nc = _cached["nc"]
    in_maps = _host_prep(X_query, X_fit)
    out = run_bass_kernel_spmd(nc, in_maps, list(range(NCORES)))
    return _gather(out.results)


# revision 51
# speedup vs baseline: 1.0267x; 1.0013x over previous
"""Batched KDE kernel for Trainium2 (8 NeuronCores, SPMD).

Problem: out[b, n] = sum_m exp(-||Xq[b,n] - Xf[b,m]||^2 / bw[b])
  with Silverman bandwidth bw[b] from Xf; b=4, n=m=4096, d=32.

Sharding: data-parallel over batch b (4 batches x 2 shards of query rows
= 8 cores). Each core handles n_shard=2048 query rows against the full
m=4096 fit set of its batch.

Device algorithm (per core), raw Bass with manual semaphores:
  psum[n, m] = 2*dot - nmu2 via TWO bf16 K=128 matmuls per 512-col chunk
  (bf16 streams at 1 col/cycle; f32 values are split into bf16 pieces
  x = x1+x2+x3+O(2^-24); Q = 2*Xq^T, f = Xf^T, s = f32(f^2)):
    mmA: lhsT=[q1; q1; q1; -1]   rhs=[f1; f2; f3; s1]
    mmB: lhsT=[q2; q2; q3; -1]   rhs=[f1; f2; f1; s2]
  ScalarE activation computes exp(psum/bw - nx2/bw) with a fused
  per-partition accumulate (accum_out) -> the sum over m. ACT is the
  bottleneck (~2.05us per 2048-col group); the schedule keeps its exp
  stream dense and starts it early:
    - bias (-nx2/bw) and scale (1/bw) are host-computed, riding in the
      first scalar-queue DMA (f32 bytes bitcast into the bf16 blob)
    - inputs live in ONE dram blob ordered by first-use, split across
      BOTH HWDGE queues (sync + scalar) for ~2x head bandwidth
    - -1 rows are baked into the blob (no memsets ahead of the PE)
    - PE warmup matmuls on garbage SBUF start immediately so the HAM
      clock gate is released before the first real group
    - exp output is written back IN PLACE to the psum region it reads
      (discarded data; ScalarE->PSUM is the faster port)
    - the final acc->res reduction is split in two so only half of it
      sits after the last exp group
  NOTE: engines run in relaxed ordering mode — any same-engine RAW
  (e.g. vector reduce then add on the same column) needs an explicit
  semaphore between producer and consumer.
Host does sharding/layout/packing plus the 4 scalar bandwidth values and
query norms (global quantile needs a sort; both are O(input) prep).
"""

import numpy as np

B, N, M, D = 4, 4096, 4096, 32
NCORES = 8
SHARDS_PER_BATCH = NCORES // B  # 2
NSHARD = N // SHARDS_PER_BATCH  # 2048
NT = NSHARD // 128  # 16 n-tiles per core
MCHUNK = 512  # matmul free-dim chunk (one psum bank)
ACT_FD = 2048  # activation free dim (4 psum banks)
NG = NT * (M // ACT_FD)  # 32 matmul/exp groups

# blob column offsets (bf16 cols, 64-aligned), ordered by first use and
# grouped into contiguous per-queue transfer ranges. q-piece regions hold
# only 96 data rows (the -1 rows are memset on-device); per-tile lhsT
# pairs [la_t | lb_t] are contiguous so each tile is one small transfer.
OFF_Q0 = 0  # la_t0 (128) | lb_t0 (128)
OFF_CA = 256  # ra m[0:512)
OFF_DA = 768  # rb m[0:512)
OFF_M = 1280  # meta: f32 [128,17] bitcast -> 34 bf16 cols (padded to 64)
OFF_CB = 1344  # ra m[512:1024)
OFF_DB = 1856  # rb m[512:1024)
OFF_G = 2368  # ra m[1024:2048)
OFF_H = 3392  # rb m[1024:2048)
OFF_QT = 4416  # tiles 1..15: [la_t | lb_t] x 256 cols each
OFF_I = 8256  # ra m[2048:4096) (2048)
OFF_J = 10304  # rb m[2048:4096) (2048)
BLOB_W = 12352

_cached = {}


def _la_off(t):
    if t == 0:
        return OFF_Q0
    return OFF_QT + (t - 1) * 256


def _lb_off(t):
    if t == 0:
        return OFF_Q0 + 128
    return OFF_QT + (t - 1) * 256 + 128


def _ra_off(c):  # c = m-col / 512, 0..7
    if c == 0:
        return OFF_CA
    if c == 1:
        return OFF_CB
    if c <= 3:
        return OFF_G + (c - 2) * 512
    return OFF_I + (c - 4) * 512


def _rb_off(c):
    if c == 0:
        return OFF_DA
    if c == 1:
        return OFF_DB
    if c <= 3:
        return OFF_H + (c - 2) * 512
    return OFF_J + (c - 4) * 512


def _build_program():
    import concourse.bass as bass
    import concourse.mybir as mybir
    from contextlib import ExitStack

    nc = bass.Bass()
    f32 = mybir.dt.float32
    bf16 = mybir.dt.bfloat16

    blob = nc.declare_dram_parameter("blob", [128, BLOB_W], bf16, isOutput=False)
    res = nc.declare_dram_parameter("res", [128, NT], f32, isOutput=True)

    with ExitStack() as ctx:
        msb = ctx.enter_context(nc.sbuf_tensor([128, BLOB_W], bf16))
        escr0 = ctx.enter_context(nc.sbuf_tensor([128, ACT_FD], bf16))
        escr1 = ctx.enter_context(nc.sbuf_tensor([128, ACT_FD], bf16))
        escr = [escr0, escr1]
        # slot 2t+h per group; slot NG = split-off first half of group 0
        acc = ctx.enter_context(nc.sbuf_tensor([128, NG + 1], f32))
        res_sb = ctx.enter_context(nc.sbuf_tensor([128, NT], f32))
        warmT = ctx.enter_context(nc.sbuf_tensor([128, 1], f32))
        wscr = ctx.enter_context(nc.sbuf_tensor([128, 640], bf16))
        ps0 = ctx.enter_context(nc.psum_tensor("ps0", [128, ACT_FD], f32))
        ps1 = ctx.enter_context(nc.psum_tensor("ps1", [128, ACT_FD], f32))
        ps = [ps0, ps1]

        s_q0 = ctx.enter_context(nc.semaphore("s_q0"))
        s_cd = ctx.enter_context(nc.semaphore("s_cd"))
        s_m = ctx.enter_context(nc.semaphore("s_m"))
        s_g = ctx.enter_context(nc.semaphore("s_g"))
        s_h2 = ctx.enter_context(nc.semaphore("s_h2"))
        s_t1 = ctx.enter_context(nc.semaphore("s_t1"))
        s_t2 = ctx.enter_context(nc.semaphore("s_t2"))
        s_t36 = ctx.enter_context(nc.semaphore("s_t36"))
        s_t715 = ctx.enter_context(nc.semaphore("s_t715"))
        s_i = ctx.enter_context(nc.semaphore("s_i"))
        s_j = ctx.enter_context(nc.semaphore("s_j"))
        s_ms = ctx.enter_context(nc.semaphore("s_ms"))
        s_pe = ctx.enter_context(nc.semaphore("s_pe"))
        s_act = ctx.enter_context(nc.semaphore("s_act"))
        s_v1 = ctx.enter_context(nc.semaphore("s_v1"))
        s_v2 = ctx.enter_context(nc.semaphore("s_v2"))
        sem_out = ctx.enter_context(nc.semaphore("sem_out"))
        block = ctx.enter_context(nc.Block())

        meta32 = msb[:, OFF_M : OFF_M + 34].bitcast(f32)  # [128, 17]
        # meta32[:, t] = -nx2/bw for tile t; 1/bw is folded into the
        # matmul operands, so activations use scale=1.0 (immediate)

        @block.sync
        def _(sync):
            sync.dma_start(
                msb[0:96, OFF_Q0:OFF_CA], blob[0:96, OFF_Q0:OFF_CA]
            ).then_inc(s_q0, 16)
            sync.dma_start(msb[:, OFF_CA:OFF_M], blob[:, OFF_CA:OFF_M]).then_inc(
                s_cd, 16
            )
            sync.dma_start(msb[:, OFF_G:OFF_H], blob[:, OFF_G:OFF_H]).then_inc(
                s_g, 16
            )
            sync.dma_start(
                msb[0:96, OFF_QT : OFF_QT + 256], blob[0:96, OFF_QT : OFF_QT + 256]
            ).then_inc(s_t1, 16)
            sync.dma_start(
                msb[0:96, OFF_QT + 512 : OFF_QT + 1536],
                blob[0:96, OFF_QT + 512 : OFF_QT + 1536],
            ).then_inc(s_t36, 16)
            sync.dma_start(
                msb[0:96, OFF_QT + 256 : OFF_QT + 512],
                blob[0:96, OFF_QT + 256 : OFF_QT + 512],
            ).then_inc(s_t2, 16)
            sync.dma_start(
                msb[0:96, OFF_QT + 1536 : OFF_I], blob[0:96, OFF_QT + 1536 : OFF_I]
            ).then_inc(s_t715, 16)
            sync.dma_start(msb[:, OFF_I:OFF_J], blob[:, OFF_I:OFF_J]).then_inc(
                s_i, 16
            )
            sync.dma_start(msb[:, OFF_J:BLOB_W], blob[:, OFF_J:BLOB_W]).then_inc(
                s_j, 16
            )
            sync.wait_ge(s_v1, 1)
            sync.dma_start(res[:, 0:8], res_sb[:, 0:8]).then_inc(sem_out, 16)
            sync.wait_ge(s_v2, 1)
            # no completion wait: the NEFF teardown drains the DMA queue;
            # skipping it lets the block exit overlap the HBM write receipt
            sync.dma_start(res[:, 8:16], res_sb[:, 8:16]).then_inc(sem_out, 16)

        @block.vector
        def _(vector):
            # -1 rows (96:128) of the q-piece regions, off the DMA path
            nc.vector.memset(msb[96:128, OFF_Q0:OFF_CA], -1.0)
            nc.vector.memset(msb[96:128, OFF_QT:OFF_I], -1.0).then_inc(s_ms, 1)
            # split final reduction: tiles 0-7 as soon as their h=1 groups
            # are done, tiles 8-15 after the last group
            vector.wait_ge(s_act, 1 + 23)
            nc.vector.tensor_reduce(
                res_sb[:, 0:8],
                acc[:, 0:16].rearrange("p (t h) -> p t h", h=2),
                axis=mybir.AxisListType.X,
                op=mybir.AluOpType.add,
            ).then_inc(s_v1, 1)
            vector.wait_ge(s_act, 1 + 31)
            nc.vector.tensor_reduce(
                res_sb[:, 8:16],
                acc[:, 16:32].rearrange("p (t h) -> p t h", h=2),
                axis=mybir.AxisListType.X,
                op=mybir.AluOpType.add,
            ).then_inc(s_v2, 1)

        @block.scalar
        def _(scalar):
            # second HWDGE queue: scalar-issued DMAs interleave with the
            # sync queue on the shared port; order both queues by first use
            scalar.dma_start(msb[:, OFF_M:OFF_G], blob[:, OFF_M:OFF_G]).then_inc(
                s_m, 16
            )
            scalar.dma_start(msb[:, OFF_H:OFF_QT], blob[:, OFF_H:OFF_QT]).then_inc(
                s_h2, 16
            )
            # fire the exp table-set load; operands are garbage (meta not
            # yet DMA'd) but the output is discarded
            nc.scalar.activation(
                warmT[:],
                warmT[:],
                mybir.ActivationFunctionType.Exp,
                bias=meta32[:, 0:1],
            )
            scalar.wait_ge(s_m, 16)
            # uniform 2048-col groups; s_pe = g+1 when group g's psum is
            # filled, s_act = g+1 when its exp+accumulate is done
            for g in range(NG):
                t = g % NT
                slot = 2 * t + (g // NT)
                scalar.wait_ge(s_pe, g + 1)
                # direct InstActivation: bias/scale as immediates (the
                # bias is folded into mmB, the scale into the operands) —
                # saves two per-instruction operand-AP fetches (~180ns)
                nc.scalar.add_instruction(
                    mybir.InstActivation(
                        name=nc.get_next_instruction_name(),
                        func=mybir.ActivationFunctionType.Exp,
                        ins=[
                            nc.scalar.lower_ap(ps[g % 2][:]),
                            mybir.ImmediateValue(
                                dtype=mybir.dt.float32, value=0.0
                            ),
                            mybir.ImmediateValue(
                                dtype=mybir.dt.float32, value=1.0
                            ),
                            mybir.ImmediateValue(
                                dtype=mybir.dt.float32, value=0.0
                            ),
                        ],
                        outs=[
                            nc.scalar.lower_ap(escr[g % 2][:]),
                            nc.scalar.lower_ap(acc[:, slot : slot + 1]),
                        ],
                    )
                ).then_inc(s_act, 1)

        @block.tensor
        def _(tensor):
            # warm the PE clock (HAM) with dummy matmuls on garbage SBUF so
            # group 0 runs at 2.4 GHz; ps0 is overwritten by group 0
            for _w in range(10):
                nc.tensor.matmul(
                    ps0[:, 0:MCHUNK],
                    wscr[:, 0:128],
                    wscr[:, 128:640],
                    start=True,
                    stop=True,
                )
            for g in range(NG):
                t = g % NT
                h = g // NT
                pg = ps[g % 2]
                la = msb[:, _la_off(t) : _la_off(t) + 128]
                lb = msb[:, _lb_off(t) : _lb_off(t) + 128]
                if g == 0:
                    # chunk 1's data (scalar queue) usually lands before
                    # chunk 0's (sync queue) — run the c1 pair first
                    tensor.wait_ge(s_ms, 1)
                    tensor.wait_ge(s_q0, 16)
                    tensor.wait_ge(s_m, 16)
                    nc.tensor.matmul(
                        pg[:, MCHUNK : 2 * MCHUNK],
                        la,
                        msb[:, _ra_off(1) : _ra_off(1) + MCHUNK],
                        start=True,
                        stop=False,
                    )
                    nc.tensor.matmul(
                        pg[:, MCHUNK : 2 * MCHUNK],
                        lb,
                        msb[:, _rb_off(1) : _rb_off(1) + MCHUNK],
                        start=False,
                        stop=True,
                    )
                    tensor.wait_ge(s_cd, 16)
                    nc.tensor.matmul(
                        pg[:, 0:MCHUNK],
                        la,
                        msb[:, _ra_off(0) : _ra_off(0) + MCHUNK],
                        start=True,
                        stop=False,
                    )
                    nc.tensor.matmul(
                        pg[:, 0:MCHUNK],
                        lb,
                        msb[:, _rb_off(0) : _rb_off(0) + MCHUNK],
                        start=False,
                        stop=True,
                    )
                    tensor.wait_ge(s_g, 16)
                    for c in (2, 3):
                        nc.tensor.matmul(
                            pg[:, c * MCHUNK : (c + 1) * MCHUNK],
                            la,
                            msb[:, _ra_off(c) : _ra_off(c) + MCHUNK],
                            start=True,
                            stop=False,
                        )
                    tensor.wait_ge(s_h2, 16)
                    for c in (2, 3):
                        mm = nc.tensor.matmul(
                            pg[:, c * MCHUNK : (c + 1) * MCHUNK],
                            lb,
                            msb[:, _rb_off(c) : _rb_off(c) + MCHUNK],
                            start=False,
                            stop=True,
                        )
                        if c == 3:
                            mm.then_inc(s_pe, 1)
                    continue
                if g == 1:
                    tensor.wait_ge(s_t1, 16)
                if g == 2:
                    tensor.wait_ge(s_t2, 16)
                if g == 3:
                    tensor.wait_ge(s_t36, 16)
                if g == 7:
                    tensor.wait_ge(s_t715, 16)
                if g == 16:
                    tensor.wait_ge(s_i, 16)
                if g >= 2:
                    tensor.wait_ge(s_act, g - 1)
                for j in range(4):
                    c = 4 * h + j
                    nc.tensor.matmul(
                        pg[:, j * MCHUNK : (j + 1) * MCHUNK],
                        la,
                        msb[:, _ra_off(c) : _ra_off(c) + MCHUNK],
                        start=True,
                        stop=False,
                    )
                if g == 16:
                    tensor.wait_ge(s_j, 16)
                for j in range(4):
                    c = 4 * h + j
                    mm = nc.tensor.matmul(
                        pg[:, j * MCHUNK : (j + 1) * MCHUNK],
                        lb,
                        msb[:, _rb_off(c) : _rb_off(c) + MCHUNK],
                        start=False,
                        stop=True,
                    )
                    if j == 3:
                        mm.then_inc(s_pe, 1)

    return nc


def _bf16_split3(x):
    import ml_dtypes

    bf = ml_dtypes.bfloat16
    x = x.astype(np.float32)
    p1 = x.astype(bf)
    rem = x - p1.astype(np.float32)
    p2 = rem.astype(bf)
    rem2 = rem - p2.astype(np.float32)
    p3 = rem2.astype(bf)
    return p1, p2, p3


def _bandwidth_np(X_fit):
    # mirror of reference._bandwidth (Silverman-style)
    b, n, d = X_fit.shape
    flat = np.asarray(X_fit, dtype=np.float64).reshape(-1)
    q = np.quantile(flat, 0.75) - np.quantile(flat, 0.25)
    std = np.std(np.asarray(X_fit, dtype=np.float64).reshape(b, -1), axis=1, ddof=1)
    return (0.9 * np.minimum(std, q / 1.34) / (n**0.2)).astype(np.float32)


def _host_prep(X_query, X_fit):
    import ml_dtypes

    bf = ml_dtypes.bfloat16
    X_query = np.asarray(X_query, dtype=np.float32)
    X_fit = np.asarray(X_fit, dtype=np.float32)
    bw = _bandwidth_np(X_fit)  # [B]

    in_maps = []
    for c in range(NCORES):
        b = c // SHARDS_PER_BATCH
        s = c % SHARDS_PER_BATCH
        XQ = X_query[b, s * NSHARD : (s + 1) * NSHARD]  # [2048, 32]
        XF = X_fit[b]  # [4096, 32]
        inv_bw = np.float32(1.0) / bw[b]

        # permuted queries: tile t / partition p handles query row p*NT + t.
        # 1/bw is folded into the operands so the activation runs with an
        # immediate scale of 1.0 (no per-instruction scale-AP fetch).
        XQp = XQ.reshape(128, NT, D).transpose(1, 0, 2).reshape(NSHARD, D)
        Q = np.ascontiguousarray(
            (2.0 * np.float64(inv_bw) * XQp.T.astype(np.float64)).astype(np.float32)
        )  # [32, 2048]
        q1, q2, q3 = _bf16_split3(Q)
        FT = np.ascontiguousarray(XF.T.astype(np.float32))  # [32, 4096]
        f1, f2, f3 = _bf16_split3(FT)
        sqr = (
            FT.astype(np.float64) ** 2 * np.float64(inv_bw)
        ).astype(np.float32)  # |f|^2 / bw
        s1, s2, _s3 = _bf16_split3(sqr)

        # the per-row bias (-|q|^2/bw) rides inside mmB as two bf16 lhsT
        # rows against rhs rows of ones, so the activation needs neither a
        # bias nor a scale operand (both immediates); q3 keeps 30 of 32
        # dims to make room (the dropped q3*f1 tail is ~5e-4 on the arg)
        nx2 = (XQ.reshape(128, NT, D).astype(np.float64) ** 2).sum(-1)
        br = (
            ((-nx2) * np.float64(inv_bw)).astype(np.float32).T.reshape(1, NSHARD)
        )  # col t*128+a = bias for tile t, partition a
        b1 = br.astype(bf)
        b2 = (br - b1.astype(np.float32)).astype(bf)
        ones2 = np.ones((2, M), dtype=bf)

        la = np.concatenate([q1, q1, q1], axis=0)  # [96, 2048]
        lb = np.concatenate([q2, q2, b1, b2, q3[:30]], axis=0)
        ra = np.concatenate([f1, f2, f3, s1], axis=0)  # [128, 4096]
        rb = np.concatenate([f1, f2, ones2, f1[:30], s2], axis=0)

        meta = np.empty((128, 17), dtype=np.float32)
        meta[:, 0:16] = (-nx2 * np.float64(inv_bw)).astype(np.float32)
        meta[:, 16] = inv_bw

        blob = np.zeros((128, BLOB_W), dtype=bf)
        for t in range(NT):
            lo = OFF_Q0 if t == 0 else OFF_QT + (t - 1) * 256
            blob[0:96, lo : lo + 128] = la[:, t * 128 : (t + 1) * 128]
            blob[0:96, lo + 128 : lo + 256] = lb[:, t * 128 : (t + 1) * 128]
        blob[:, OFF_CA : OFF_CA + 512] = ra[:, 0:512]
        blob[:, OFF_DA : OFF_DA + 512] = rb[:, 0:512]
        blob[:, OFF_M : OFF_M + 34] = meta.view(np.uint16).view(bf)  # raw bytes
        blob[:, OFF_CB : OFF_CB + 512] = ra[:, 512:1024]
        blob[:, OFF_DB : OFF_DB + 512] = rb[:, 512:1024]
        blob[:, OFF_G : OFF_G + 1024] = ra[:, 1024:2048]
        blob[:, OFF_H : OFF_H + 1024] = rb[:, 1024:2048]
        blob[:, OFF_I : OFF_I + 2048] = ra[:, 2048:4096]
        blob[:, OFF_J : OFF_J + 2048] = rb[:, 2048:4096]

        in_maps.append({"blob": blob})
    return in_maps


def _gather(results):
    out = np.empty((B, N), dtype=np.float32)
    for c in range(NCORES):
        b = c // SHARDS_PER_BATCH
        s = c % SHARDS_PER_BATCH
        res = np.asarray(results[c]["res"], dtype=np.float32)  # [128, 16]
        out[b, s * NSHARD : (s + 1) * NSHARD] = res.reshape(NSHARD)
    return out


def kernel(X_query, X_fit):
    from concourse.bass_utils import run_bass_kernel_spmd

    if "nc" not in _cached:
        _cached["nc"] = _build_program()
    nc = _cached["nc"]
    in_maps = _host_prep(X_query, X_fit)
    out = run_bass_kernel_spmd(nc, in_maps, list(range(NCORES)))
    return _gather(out.results)


# revision 52
# speedup vs baseline: 1.0658x; 1.0381x over previous
"""Batched KDE kernel for Trainium2 (8 NeuronCores, SPMD).

Problem: out[b, n] = sum_m exp(-||Xq[b,n] - Xf[b,m]||^2 / bw[b])
  with Silverman bandwidth bw[b] from Xf; b=4, n=m=4096, d=32.

Sharding: data-parallel over batch b (4 batches x 2 shards of query rows
= 8 cores). Each core handles n_shard=2048 query rows against the full
m=4096 fit set of its batch.

Device algorithm (per core), raw Bass with manual semaphores:
  psum[n, m] = 2*dot - nmu2 via TWO bf16 K=128 matmuls per 512-col chunk
  (bf16 streams at 1 col/cycle; f32 values are split into bf16 pieces
  x = x1+x2+x3+O(2^-24); Q = 2*Xq^T, f = Xf^T, s = f32(f^2)):
    mmA: lhsT=[q1; q1; q1; -1]   rhs=[f1; f2; f3; s1]
    mmB: lhsT=[q2; q2; q3; -1]   rhs=[f1; f2; f1; s2]
  ScalarE activation computes exp(psum/bw - nx2/bw) with a fused
  per-partition accumulate (accum_out) -> the sum over m. ACT is the
  bottleneck (~2.05us per 2048-col group); the schedule keeps its exp
  stream dense and starts it early:
    - bias (-nx2/bw) and scale (1/bw) are host-computed, riding in the
      first scalar-queue DMA (f32 bytes bitcast into the bf16 blob)
    - inputs live in ONE dram blob ordered by first-use, split across
      BOTH HWDGE queues (sync + scalar) for ~2x head bandwidth
    - -1 rows are baked into the blob (no memsets ahead of the PE)
    - PE warmup matmuls on garbage SBUF start immediately so the HAM
      clock gate is released before the first real group
    - exp output is written back IN PLACE to the psum region it reads
      (discarded data; ScalarE->PSUM is the faster port)
    - the final acc->res reduction is split in two so only half of it
      sits after the last exp group
  NOTE: engines run in relaxed ordering mode — any same-engine RAW
  (e.g. vector reduce then add on the same column) needs an explicit
  semaphore between producer and consumer.
Host does sharding/layout/packing plus the 4 scalar bandwidth values and
query norms (global quantile needs a sort; both are O(input) prep).
"""

import numpy as np

B, N, M, D = 4, 4096, 4096, 32
NCORES = 8
SHARDS_PER_BATCH = NCORES // B  # 2
NSHARD = N // SHARDS_PER_BATCH  # 2048
NT = NSHARD // 128  # 16 n-tiles per core
MCHUNK = 512  # matmul free-dim chunk (one psum bank)
ACT_FD = 2048  # activation free dim (4 psum banks)
NG = NT * (M // ACT_FD)  # 32 matmul/exp groups

# blob column offsets (bf16 cols, 64-aligned), ordered by first use and
# grouped into contiguous per-queue transfer ranges. q-piece regions hold
# only 96 data rows (the -1 rows are memset on-device); per-tile lhsT
# pairs [la_t | lb_t] are contiguous so each tile is one small transfer.
OFF_Q0 = 0  # la_t0 (128) | lb_t0 (128)
OFF_CA = 256  # ra m[0:512)
OFF_DA = 768  # rb m[0:512)
OFF_M = 1280  # meta: f32 [128,17] bitcast -> 34 bf16 cols (padded to 64)
OFF_CB = 1344  # ra m[512:1024)
OFF_DB = 1856  # rb m[512:1024)
OFF_G = 2368  # ra m[1024:2048)
OFF_H = 3392  # rb m[1024:2048)
OFF_QT = 4416  # tiles 1..15: [la_t | lb_t] x 256 cols each
OFF_I = 8256  # ra m[2048:4096) (2048)
OFF_J = 10304  # rb m[2048:4096) (2048)
BLOB_W = 12352

_cached = {}


def _la_off(t):
    if t == 0:
        return OFF_Q0
    return OFF_QT + (t - 1) * 256


def _lb_off(t):
    if t == 0:
        return OFF_Q0 + 128
    return OFF_QT + (t - 1) * 256 + 128


def _ra_off(c):  # c = m-col / 512, 0..7
    if c == 0:
        return OFF_CA
    if c == 1:
        return OFF_CB
    if c <= 3:
        return OFF_G + (c - 2) * 512
    return OFF_I + (c - 4) * 512


def _rb_off(c):
    if c == 0:
        return OFF_DA
    if c == 1:
        return OFF_DB
    if c <= 3:
        return OFF_H + (c - 2) * 512
    return OFF_J + (c - 4) * 512


def _build_program():
    import concourse.bass as bass
    import concourse.mybir as mybir
    from contextlib import ExitStack

    nc = bass.Bass()
    f32 = mybir.dt.float32
    bf16 = mybir.dt.bfloat16

    blob = nc.declare_dram_parameter("blob", [128, BLOB_W], bf16, isOutput=False)
    res = nc.declare_dram_parameter("res", [128, NT], f32, isOutput=True)

    with ExitStack() as ctx:
        msb = ctx.enter_context(nc.sbuf_tensor([128, BLOB_W], bf16))
        escr0 = ctx.enter_context(nc.sbuf_tensor([128, ACT_FD], bf16))
        escr1 = ctx.enter_context(nc.sbuf_tensor([128, ACT_FD], bf16))
        escr = [escr0, escr1]
        # slot 2t+h per group; slot NG = split-off first half of group 0
        acc = ctx.enter_context(nc.sbuf_tensor([128, NG + 1], f32))
        res_sb = ctx.enter_context(nc.sbuf_tensor([128, NT], f32))
        warmT = ctx.enter_context(nc.sbuf_tensor([128, 1], f32))
        wscr = ctx.enter_context(nc.sbuf_tensor([128, 640], bf16))
        ps0 = ctx.enter_context(nc.psum_tensor("ps0", [128, ACT_FD], f32))
        ps1 = ctx.enter_context(nc.psum_tensor("ps1", [128, ACT_FD], f32))
        ps = [ps0, ps1]

        s_q0 = ctx.enter_context(nc.semaphore("s_q0"))
        s_cd = ctx.enter_context(nc.semaphore("s_cd"))
        s_m = ctx.enter_context(nc.semaphore("s_m"))
        s_g = ctx.enter_context(nc.semaphore("s_g"))
        s_h2 = ctx.enter_context(nc.semaphore("s_h2"))
        s_t1 = ctx.enter_context(nc.semaphore("s_t1"))
        s_t2 = ctx.enter_context(nc.semaphore("s_t2"))
        s_t36 = ctx.enter_context(nc.semaphore("s_t36"))
        s_t715 = ctx.enter_context(nc.semaphore("s_t715"))
        s_i = ctx.enter_context(nc.semaphore("s_i"))
        s_j = ctx.enter_context(nc.semaphore("s_j"))
        s_ms = ctx.enter_context(nc.semaphore("s_ms"))
        s_pe = ctx.enter_context(nc.semaphore("s_pe"))
        s_act = ctx.enter_context(nc.semaphore("s_act"))
        s_v1 = ctx.enter_context(nc.semaphore("s_v1"))
        s_v2 = ctx.enter_context(nc.semaphore("s_v2"))
        sem_out = ctx.enter_context(nc.semaphore("sem_out"))
        block = ctx.enter_context(nc.Block())

        meta32 = msb[:, OFF_M : OFF_M + 34].bitcast(f32)  # [128, 17]
        # meta32[:, t] = -nx2/bw for tile t; 1/bw is folded into the
        # matmul operands, so activations use scale=1.0 (immediate)

        @block.sync
        def _(sync):
            sync.dma_start(
                msb[0:96, OFF_Q0:OFF_CA], blob[0:96, OFF_Q0:OFF_CA]
            ).then_inc(s_q0, 16)
            sync.dma_start(msb[:, OFF_CA:OFF_M], blob[:, OFF_CA:OFF_M]).then_inc(
                s_cd, 16
            )
            sync.dma_start(msb[:, OFF_G:OFF_H], blob[:, OFF_G:OFF_H]).then_inc(
                s_g, 16
            )
            sync.dma_start(
                msb[0:96, OFF_QT : OFF_QT + 256], blob[0:96, OFF_QT : OFF_QT + 256]
            ).then_inc(s_t1, 16)
            sync.dma_start(
                msb[0:96, OFF_QT + 512 : OFF_QT + 1536],
                blob[0:96, OFF_QT + 512 : OFF_QT + 1536],
            ).then_inc(s_t36, 16)
            sync.dma_start(
                msb[0:96, OFF_QT + 256 : OFF_QT + 512],
                blob[0:96, OFF_QT + 256 : OFF_QT + 512],
            ).then_inc(s_t2, 16)
            sync.dma_start(
                msb[0:96, OFF_QT + 1536 : OFF_I], blob[0:96, OFF_QT + 1536 : OFF_I]
            ).then_inc(s_t715, 16)
            sync.dma_start(msb[:, OFF_I:OFF_J], blob[:, OFF_I:OFF_J]).then_inc(
                s_i, 16
            )
            sync.dma_start(msb[:, OFF_J:BLOB_W], blob[:, OFF_J:BLOB_W]).then_inc(
                s_j, 16
            )
            sync.wait_ge(s_v1, 1)
            sync.dma_start(res[:, 0:8], res_sb[:, 0:8]).then_inc(sem_out, 16)
            sync.wait_ge(s_v2, 1)
            # no completion wait: the NEFF teardown drains the DMA queue;
            # skipping it lets the block exit overlap the HBM write receipt
            sync.dma_start(res[:, 8:16], res_sb[:, 8:16]).then_inc(sem_out, 16)

        @block.vector
        def _(vector):
            # -1 rows (96:128) of the q-piece regions, off the DMA path
            nc.vector.memset(msb[96:128, OFF_Q0:OFF_CA], -1.0)
            nc.vector.memset(msb[96:128, OFF_QT:OFF_I], -1.0).then_inc(s_ms, 1)
            # split final reduction: tiles 0-7 as soon as their h=1 groups
            # are done, tiles 8-15 after the last group
            vector.wait_ge(s_act, 1 + 23)
            nc.vector.tensor_reduce(
                res_sb[:, 0:8],
                acc[:, 0:16].rearrange("p (t h) -> p t h", h=2),
                axis=mybir.AxisListType.X,
                op=mybir.AluOpType.add,
            ).then_inc(s_v1, 1)
            vector.wait_ge(s_act, 1 + 31)
            nc.vector.tensor_reduce(
                res_sb[:, 8:16],
                acc[:, 16:32].rearrange("p (t h) -> p t h", h=2),
                axis=mybir.AxisListType.X,
                op=mybir.AluOpType.add,
            ).then_inc(s_v2, 1)

        @block.scalar
        def _(scalar):
            # second HWDGE queue: scalar-issued DMAs interleave with the
            # sync queue on the shared port; order both queues by first use
            scalar.dma_start(msb[:, OFF_M:OFF_G], blob[:, OFF_M:OFF_G]).then_inc(
                s_m, 16
            )
            scalar.dma_start(msb[:, OFF_H:OFF_QT], blob[:, OFF_H:OFF_QT]).then_inc(
                s_h2, 16
            )
            # fire the exp table-set load; operands are garbage (meta not
            # yet DMA'd) but the output is discarded
            nc.scalar.activation(
                warmT[:],
                warmT[:],
                mybir.ActivationFunctionType.Exp,
                bias=meta32[:, 0:1],
            )
            scalar.wait_ge(s_m, 16)
            # uniform 2048-col groups; s_pe = g+1 when group g's psum is
            # filled, s_act = g+1 when its exp+accumulate is done
            for g in range(NG):
                t = g % NT
                slot = 2 * t + (g // NT)
                scalar.wait_ge(s_pe, g + 1)
                # direct InstActivation: bias/scale as immediates (the
                # bias is folded into mmB, the scale into the operands) —
                # saves two per-instruction operand-AP fetches (~180ns)
                nc.scalar.add_instruction(
                    mybir.InstActivation(
                        name=nc.get_next_instruction_name(),
                        func=mybir.ActivationFunctionType.Exp,
                        ins=[
                            nc.scalar.lower_ap(ps[g % 2][:]),
                            mybir.ImmediateValue(
                                dtype=mybir.dt.float32, value=0.0
                            ),
                            mybir.ImmediateValue(
                                dtype=mybir.dt.float32, value=1.0
                            ),
                            mybir.ImmediateValue(
                                dtype=mybir.dt.float32, value=0.0
                            ),
                        ],
                        outs=[
                            nc.scalar.lower_ap(escr[g % 2][:]),
                            nc.scalar.lower_ap(acc[:, slot : slot + 1]),
                        ],
                    )
                ).then_inc(s_act, 1)

        @block.tensor
        def _(tensor):
            # warm the PE clock (HAM) with dummy matmuls on garbage SBUF so
            # group 0 runs at 2.4 GHz; ps0 is overwritten by group 0
            for _w in range(10):
                nc.tensor.matmul(
                    ps0[:, 0:MCHUNK],
                    wscr[:, 0:128],
                    wscr[:, 128:640],
                    start=True,
                    stop=True,
                )
            for g in range(NG):
                t = g % NT
                h = g // NT
                pg = ps[g % 2]
                la = msb[:, _la_off(t) : _la_off(t) + 128]
                lb = msb[:, _lb_off(t) : _lb_off(t) + 128]
                if g == 0:
                    # chunk 1's data (scalar queue) usually lands before
                    # chunk 0's (sync queue) — run the c1 pair first
                    tensor.wait_ge(s_ms, 1)
                    tensor.wait_ge(s_q0, 16)
                    tensor.wait_ge(s_m, 16)
                    nc.tensor.matmul(
                        pg[:, MCHUNK : 2 * MCHUNK],
                        la,
                        msb[:, _ra_off(1) : _ra_off(1) + MCHUNK],
                        start=True,
                        stop=False,
                    )
                    nc.tensor.matmul(
                        pg[:, MCHUNK : 2 * MCHUNK],
                        lb,
                        msb[:, _rb_off(1) : _rb_off(1) + MCHUNK],
                        start=False,
                        stop=True,
                    )
                    tensor.wait_ge(s_cd, 16)
                    nc.tensor.matmul(
                        pg[:, 0:MCHUNK],
                        la,
                        msb[:, _ra_off(0) : _ra_off(0) + MCHUNK],
                        start=True,
                        stop=False,
                    )
                    nc.tensor.matmul(
                        pg[:, 0:MCHUNK],
                        lb,
                        msb[:, _rb_off(0) : _rb_off(0) + MCHUNK],
                        start=False,
                        stop=True,
                    )
                    tensor.wait_ge(s_g, 16)
                    for c in (2, 3):
                        nc.tensor.matmul(
                            pg[:, c * MCHUNK : (c + 1) * MCHUNK],
                            la,
                            msb[:, _ra_off(c) : _ra_off(c) + MCHUNK],
                            start=True,
                            stop=False,
                        )
                    tensor.wait_ge(s_h2, 16)
                    for c in (2, 3):
                        mm = nc.tensor.matmul(
                            pg[:, c * MCHUNK : (c + 1) * MCHUNK],
                            lb,
                            msb[:, _rb_off(c) : _rb_off(c) + MCHUNK],
                            start=False,
                            stop=True,
                        )
                        if c == 3:
                            mm.then_inc(s_pe, 1)
                    continue
                if g == 1:
                    tensor.wait_ge(s_t1, 16)
                if g == 2:
                    tensor.wait_ge(s_t2, 16)
                if g == 3:
                    tensor.wait_ge(s_t36, 16)
                if g == 7:
                    tensor.wait_ge(s_t715, 16)
                if g == 16:
                    tensor.wait_ge(s_i, 16)
                if g >= 2:
                    tensor.wait_ge(s_act, g - 1)
                for j in range(4):
                    c = 4 * h + j
                    nc.tensor.matmul(
                        pg[:, j * MCHUNK : (j + 1) * MCHUNK],
                        la,
                        msb[:, _ra_off(c) : _ra_off(c) + MCHUNK],
                        start=True,
                        stop=False,
                    )
                if g == 16:
                    tensor.wait_ge(s_j, 16)
                for j in range(4):
                    c = 4 * h + j
                    mm = nc.tensor.matmul(
                        pg[:, j * MCHUNK : (j + 1) * MCHUNK],
                        lb,
                        msb[:, _rb_off(c) : _rb_off(c) + MCHUNK],
                        start=False,
                        stop=True,
                    )
                    if j == 2:
                        # release the group one matmul early: the ACT reads
                        # psum sequentially and reaches chunk 3's columns
                        # ~1.4us after waking, while mmB j=3 (216ns, already
                        # dispatched in-order behind this one) lands ~1.2us
                        # before that — shortens the psum-ring round trip
                        mm.then_inc(s_pe, 1)

    return nc


def _bf16_split3(x):
    import ml_dtypes

    bf = ml_dtypes.bfloat16
    x = x.astype(np.float32)
    p1 = x.astype(bf)
    rem = x - p1.astype(np.float32)
    p2 = rem.astype(bf)
    rem2 = rem - p2.astype(np.float32)
    p3 = rem2.astype(bf)
    return p1, p2, p3


def _bandwidth_np(X_fit):
    # mirror of reference._bandwidth (Silverman-style)
    b, n, d = X_fit.shape
    flat = np.asarray(X_fit, dtype=np.float64).reshape(-1)
    q = np.quantile(flat, 0.75) - np.quantile(flat, 0.25)
    std = np.std(np.asarray(X_fit, dtype=np.float64).reshape(b, -1), axis=1, ddof=1)
    return (0.9 * np.minimum(std, q / 1.34) / (n**0.2)).astype(np.float32)


def _host_prep(X_query, X_fit):
    import ml_dtypes

    bf = ml_dtypes.bfloat16
    X_query = np.asarray(X_query, dtype=np.float32)
    X_fit = np.asarray(X_fit, dtype=np.float32)
    bw = _bandwidth_np(X_fit)  # [B]

    in_maps = []
    for c in range(NCORES):
        b = c // SHARDS_PER_BATCH
        s = c % SHARDS_PER_BATCH
        XQ = X_query[b, s * NSHARD : (s + 1) * NSHARD]  # [2048, 32]
        XF = X_fit[b]  # [4096, 32]
        inv_bw = np.float32(1.0) / bw[b]

        # permuted queries: tile t / partition p handles query row p*NT + t.
        # 1/bw is folded into the operands so the activation runs with an
        # immediate scale of 1.0 (no per-instruction scale-AP fetch).
        XQp = XQ.reshape(128, NT, D).transpose(1, 0, 2).reshape(NSHARD, D)
        Q = np.ascontiguousarray(
            (2.0 * np.float64(inv_bw) * XQp.T.astype(np.float64)).astype(np.float32)
        )  # [32, 2048]
        q1, q2, q3 = _bf16_split3(Q)
        FT = np.ascontiguousarray(XF.T.astype(np.float32))  # [32, 4096]
        f1, f2, f3 = _bf16_split3(FT)
        sqr = (
            FT.astype(np.float64) ** 2 * np.float64(inv_bw)
        ).astype(np.float32)  # |f|^2 / bw
        s1, s2, _s3 = _bf16_split3(sqr)

        # the per-row bias (-|q|^2/bw) rides inside mmB as two bf16 lhsT
        # rows against rhs rows of ones, so the activation needs neither a
        # bias nor a scale operand (both immediates); q3 keeps 30 of 32
        # dims to make room (the dropped q3*f1 tail is ~5e-4 on the arg)
        nx2 = (XQ.reshape(128, NT, D).astype(np.float64) ** 2).sum(-1)
        br = (
            ((-nx2) * np.float64(inv_bw)).astype(np.float32).T.reshape(1, NSHARD)
        )  # col t*128+a = bias for tile t, partition a
        b1 = br.astype(bf)
        b2 = (br - b1.astype(np.float32)).astype(bf)
        ones2 = np.ones((2, M), dtype=bf)

        la = np.concatenate([q1, q1, q1], axis=0)  # [96, 2048]
        lb = np.concatenate([q2, q2, b1, b2, q3[:30]], axis=0)
        ra = np.concatenate([f1, f2, f3, s1], axis=0)  # [128, 4096]
        rb = np.concatenate([f1, f2, ones2, f1[:30], s2], axis=0)

        meta = np.empty((128, 17), dtype=np.float32)
        meta[:, 0:16] = (-nx2 * np.float64(inv_bw)).astype(np.float32)
        meta[:, 16] = inv_bw

        blob = np.zeros((128, BLOB_W), dtype=bf)
        for t in range(NT):
            lo = OFF_Q0 if t == 0 else OFF_QT + (t - 1) * 256
            blob[0:96, lo : lo + 128] = la[:, t * 128 : (t + 1) * 128]
            blob[0:96, lo + 128 : lo + 256] = lb[:, t * 128 : (t + 1) * 128]
        blob[:, OFF_CA : OFF_CA + 512] = ra[:, 0:512]
        blob[:, OFF_DA : OFF_DA + 512] = rb[:, 0:512]
        blob[:, OFF_M : OFF_M + 34] = meta.view(np.uint16).view(bf)  # raw bytes
        blob[:, OFF_CB : OFF_CB + 512] = ra[:, 512:1024]
        blob[:, OFF_DB : OFF_DB + 512] = rb[:, 512:1024]
        blob[:, OFF_G : OFF_G + 1024] = ra[:, 1024:2048]
        blob[:, OFF_H : OFF_H + 1024] = rb[:, 1024:2048]
        blob[:, OFF_I : OFF_I + 2048] = ra[:, 2048:4096]
        blob[:, OFF_J : OFF_J + 2048] = rb[:, 2048:4096]

        in_maps.append({"blob": blob})
    return in_maps


def _gather(results):
    out = np.empty((B, N), dtype=np.float32)
    for c in range(NCORES):
        b = c // SHARDS_PER_BATCH
        s = c % SHARDS_PER_BATCH
        res = np.asarray(results[c]["res"], dtype=np.float32)  # [128, 16]
        out[b, s * NSHARD : (s + 1) * NSHARD] = res.reshape(NSHARD)
    return out


def kernel(X_query, X_fit):
    from concourse.bass_utils import run_bass_kernel_spmd

    if "nc" not in _cached:
        _cached["nc"] = _build_program()
    nc = _cached["nc"]
    in_maps = _host_prep(X_query, X_fit)
    out = run_bass_kernel_spmd(nc, in_maps, list(range(NCORES)))
    return _gather(out.results)


# revision 54
# speedup vs baseline: 1.0876x; 1.0204x over previous
"""Batched KDE kernel for Trainium2 (8 NeuronCores, SPMD).

Problem: out[b, n] = sum_m exp(-||Xq[b,n] - Xf[b,m]||^2 / bw[b])
  with Silverman bandwidth bw[b] from Xf; b=4, n=m=4096, d=32.

Sharding: data-parallel over batch b (4 batches x 2 shards of query rows
= 8 cores). Each core handles n_shard=2048 query rows against the full
m=4096 fit set of its batch.

Device algorithm (per core), raw Bass with manual semaphores:
  psum[n, m] = 2*dot - nmu2 via TWO bf16 K=128 matmuls per 512-col chunk
  (bf16 streams at 1 col/cycle; f32 values are split into bf16 pieces
  x = x1+x2+x3+O(2^-24); Q = 2*Xq^T, f = Xf^T, s = f32(f^2)):
    mmA: lhsT=[q1; q1; q1; -1]   rhs=[f1; f2; f3; s1]
    mmB: lhsT=[q2; q2; q3; -1]   rhs=[f1; f2; f1; s2]
  ScalarE activation computes exp(psum/bw - nx2/bw) with a fused
  per-partition accumulate (accum_out) -> the sum over m. ACT is the
  bottleneck (~2.05us per 2048-col group); the schedule keeps its exp
  stream dense and starts it early:
    - bias (-nx2/bw) and scale (1/bw) are host-computed, riding in the
      first scalar-queue DMA (f32 bytes bitcast into the bf16 blob)
    - inputs live in ONE dram blob ordered by first-use, split across
      BOTH HWDGE queues (sync + scalar) for ~2x head bandwidth
    - -1 rows are baked into the blob (no memsets ahead of the PE)
    - PE warmup matmuls on garbage SBUF start immediately so the HAM
      clock gate is released before the first real group
    - exp output is written back IN PLACE to the psum region it reads
      (discarded data; ScalarE->PSUM is the faster port)
    - the final acc->res reduction is split in two so only half of it
      sits after the last exp group
  NOTE: engines run in relaxed ordering mode — any same-engine RAW
  (e.g. vector reduce then add on the same column) needs an explicit
  semaphore between producer and consumer.
Host does sharding/layout/packing plus the 4 scalar bandwidth values and
query norms (global quantile needs a sort; both are O(input) prep).
"""

import numpy as np

B, N, M, D = 4, 4096, 4096, 32
NCORES = 8
SHARDS_PER_BATCH = NCORES // B  # 2
NSHARD = N // SHARDS_PER_BATCH  # 2048
NT = NSHARD // 128  # 16 n-tiles per core
MCHUNK = 512  # matmul free-dim chunk (one psum bank)
ACT_FD = 2048  # activation free dim (4 psum banks)
NG = NT * (M // ACT_FD)  # 32 matmul/exp groups

# blob column offsets (bf16 cols, 64-aligned), ordered by first use and
# grouped into contiguous per-queue transfer ranges. q-piece regions hold
# only 96 data rows (the -1 rows are memset on-device); per-tile lhsT
# pairs [la_t | lb_t] are contiguous so each tile is one small transfer.
OFF_Q0 = 0  # la_t0 (128) | lb_t0 (128)
OFF_CA = 256  # ra m[0:512)
OFF_DA = 768  # rb m[0:512)
OFF_M = 1280  # meta: f32 [128,17] bitcast -> 34 bf16 cols (padded to 64)
OFF_CB = 1344  # ra m[512:1024)
OFF_DB = 1856  # rb m[512:1024)
OFF_G = 2368  # ra m[1024:2048)
OFF_H = 3392  # rb m[1024:2048)
OFF_QT = 4416  # tiles 1..15: [la_t | lb_t] x 256 cols each
OFF_I = 8256  # ra m[2048:4096) (2048)
OFF_J = 10304  # rb m[2048:4096) (2048)
BLOB_W = 12352

_cached = {}


def _la_off(t):
    if t == 0:
        return OFF_Q0
    return OFF_QT + (t - 1) * 256


def _lb_off(t):
    if t == 0:
        return OFF_Q0 + 128
    return OFF_QT + (t - 1) * 256 + 128


def _ra_off(c):  # c = m-col / 512, 0..7
    if c == 0:
        return OFF_CA
    if c == 1:
        return OFF_CB
    if c <= 3:
        return OFF_G + (c - 2) * 512
    return OFF_I + (c - 4) * 512


def _rb_off(c):
    if c == 0:
        return OFF_DA
    if c == 1:
        return OFF_DB
    if c <= 3:
        return OFF_H + (c - 2) * 512
    return OFF_J + (c - 4) * 512


def _build_program():
    import concourse.bass as bass
    import concourse.mybir as mybir
    from contextlib import ExitStack

    nc = bass.Bass()
    f32 = mybir.dt.float32
    bf16 = mybir.dt.bfloat16

    blob = nc.declare_dram_parameter("blob", [128, BLOB_W], bf16, isOutput=False)
    res = nc.declare_dram_parameter("res", [128, NT], f32, isOutput=True)

    with ExitStack() as ctx:
        msb = ctx.enter_context(nc.sbuf_tensor([128, BLOB_W], bf16))
        escr0 = ctx.enter_context(nc.sbuf_tensor([128, ACT_FD], bf16))
        escr1 = ctx.enter_context(nc.sbuf_tensor([128, ACT_FD], bf16))
        escr = [escr0, escr1]
        # slot 2t+h per group; slot NG = split-off first half of group 0
        acc = ctx.enter_context(nc.sbuf_tensor([128, NG + 1], f32))
        res_sb = ctx.enter_context(nc.sbuf_tensor([128, NT], f32))
        warmT = ctx.enter_context(nc.sbuf_tensor([128, 1], f32))
        wscr = ctx.enter_context(nc.sbuf_tensor([128, 640], bf16))
        ps0 = ctx.enter_context(nc.psum_tensor("ps0", [128, ACT_FD], f32))
        ps1 = ctx.enter_context(nc.psum_tensor("ps1", [128, ACT_FD], f32))
        ps = [ps0, ps1]

        s_q0 = ctx.enter_context(nc.semaphore("s_q0"))
        s_cd = ctx.enter_context(nc.semaphore("s_cd"))
        s_m = ctx.enter_context(nc.semaphore("s_m"))
        s_g = ctx.enter_context(nc.semaphore("s_g"))
        s_h2 = ctx.enter_context(nc.semaphore("s_h2"))
        s_t1 = ctx.enter_context(nc.semaphore("s_t1"))
        s_t2 = ctx.enter_context(nc.semaphore("s_t2"))
        s_t36 = ctx.enter_context(nc.semaphore("s_t36"))
        s_t715 = ctx.enter_context(nc.semaphore("s_t715"))
        s_i = ctx.enter_context(nc.semaphore("s_i"))
        s_j = ctx.enter_context(nc.semaphore("s_j"))
        s_ms = ctx.enter_context(nc.semaphore("s_ms"))
        s_pe = ctx.enter_context(nc.semaphore("s_pe"))
        s_act = ctx.enter_context(nc.semaphore("s_act"))
        s_v1 = ctx.enter_context(nc.semaphore("s_v1"))
        s_v2 = ctx.enter_context(nc.semaphore("s_v2"))
        sem_out = ctx.enter_context(nc.semaphore("sem_out"))
        block = ctx.enter_context(nc.Block())

        meta32 = msb[:, OFF_M : OFF_M + 34].bitcast(f32)  # [128, 17]
        # meta32[:, t] = -nx2/bw for tile t; 1/bw is folded into the
        # matmul operands, so activations use scale=1.0 (immediate)

        @block.sync
        def _(sync):
            sync.dma_start(
                msb[0:96, OFF_Q0:OFF_CA], blob[0:96, OFF_Q0:OFF_CA]
            ).then_inc(s_q0, 16)
            sync.dma_start(msb[:, OFF_CA:OFF_M], blob[:, OFF_CA:OFF_M]).then_inc(
                s_cd, 16
            )
            sync.dma_start(msb[:, OFF_G:OFF_H], blob[:, OFF_G:OFF_H]).then_inc(
                s_g, 16
            )
            sync.dma_start(
                msb[0:96, OFF_QT : OFF_QT + 256], blob[0:96, OFF_QT : OFF_QT + 256]
            ).then_inc(s_t1, 16)
            sync.dma_start(
                msb[0:96, OFF_QT + 512 : OFF_QT + 1536],
                blob[0:96, OFF_QT + 512 : OFF_QT + 1536],
            ).then_inc(s_t36, 16)
            sync.dma_start(
                msb[0:96, OFF_QT + 256 : OFF_QT + 512],
                blob[0:96, OFF_QT + 256 : OFF_QT + 512],
            ).then_inc(s_t2, 16)
            sync.dma_start(
                msb[0:96, OFF_QT + 1536 : OFF_I], blob[0:96, OFF_QT + 1536 : OFF_I]
            ).then_inc(s_t715, 16)
            sync.dma_start(msb[:, OFF_I:OFF_J], blob[:, OFF_I:OFF_J]).then_inc(
                s_i, 16
            )
            sync.dma_start(msb[:, OFF_J:BLOB_W], blob[:, OFF_J:BLOB_W]).then_inc(
                s_j, 16
            )
            sync.wait_ge(s_v1, 1)
            sync.dma_start(res[:, 0:8], res_sb[:, 0:8]).then_inc(sem_out, 16)
            sync.wait_ge(s_v2, 1)
            # no completion wait: the NEFF teardown drains the DMA queue;
            # skipping it lets the block exit overlap the HBM write receipt
            sync.dma_start(res[:, 8:16], res_sb[:, 8:16]).then_inc(sem_out, 16)

        @block.vector
        def _(vector):
            # -1 rows (96:128) of the q-piece regions, off the DMA path
            nc.vector.memset(msb[96:128, OFF_Q0:OFF_CA], -1.0)
            nc.vector.memset(msb[96:128, OFF_QT:OFF_I], -1.0).then_inc(s_ms, 1)
            # split final reduction: tiles 0-7 as soon as their h=1 groups
            # are done, tiles 8-15 after the last group
            vector.wait_ge(s_act, 1 + 23)
            nc.vector.tensor_reduce(
                res_sb[:, 0:8],
                acc[:, 0:16].rearrange("p (t h) -> p t h", h=2),
                axis=mybir.AxisListType.X,
                op=mybir.AluOpType.add,
            ).then_inc(s_v1, 1)
            vector.wait_ge(s_act, 1 + 31)
            nc.vector.tensor_reduce(
                res_sb[:, 8:16],
                acc[:, 16:32].rearrange("p (t h) -> p t h", h=2),
                axis=mybir.AxisListType.X,
                op=mybir.AluOpType.add,
            ).then_inc(s_v2, 1)

        @block.scalar
        def _(scalar):
            # second HWDGE queue: scalar-issued DMAs interleave with the
            # sync queue on the shared port; order both queues by first use
            scalar.dma_start(msb[:, OFF_M:OFF_G], blob[:, OFF_M:OFF_G]).then_inc(
                s_m, 16
            )
            scalar.dma_start(msb[:, OFF_H:OFF_QT], blob[:, OFF_H:OFF_QT]).then_inc(
                s_h2, 16
            )
            # fire the exp table-set load; operands are garbage (meta not
            # yet DMA'd) but the output is discarded
            nc.scalar.activation(
                warmT[:],
                warmT[:],
                mybir.ActivationFunctionType.Exp,
                bias=meta32[:, 0:1],
            )
            scalar.wait_ge(s_m, 16)
            # uniform 2048-col groups; s_pe = g+1 when group g's psum is
            # filled, s_act = g+1 when its exp+accumulate is done
            for g in range(NG):
                t = g % NT
                slot = 2 * t + (g // NT)
                scalar.wait_ge(s_pe, g + 1)
                # direct InstActivation: bias/scale as immediates (the
                # bias is folded into mmB, the scale into the operands) —
                # saves two per-instruction operand-AP fetches (~180ns)
                nc.scalar.add_instruction(
                    mybir.InstActivation(
                        name=nc.get_next_instruction_name(),
                        func=mybir.ActivationFunctionType.Exp,
                        ins=[
                            nc.scalar.lower_ap(ps[g % 2][:]),
                            mybir.ImmediateValue(
                                dtype=mybir.dt.float32, value=0.0
                            ),
                            mybir.ImmediateValue(
                                dtype=mybir.dt.float32, value=1.0
                            ),
                            mybir.ImmediateValue(
                                dtype=mybir.dt.float32, value=0.0
                            ),
                        ],
                        outs=[
                            nc.scalar.lower_ap(escr[g % 2][:]),
                            nc.scalar.lower_ap(acc[:, slot : slot + 1]),
                        ],
                    )
                ).then_inc(s_act, 1)

        @block.tensor
        def _(tensor):
            # warm the PE clock (HAM) with dummy matmuls on garbage SBUF so
            # group 0 runs at 2.4 GHz; ps0 is overwritten by group 0
            for _w in range(10):
                nc.tensor.matmul(
                    ps0[:, 0:MCHUNK],
                    wscr[:, 0:128],
                    wscr[:, 128:640],
                    start=True,
                    stop=True,
                )
            for g in range(NG):
                t = g % NT
                h = g // NT
                pg = ps[g % 2]
                la = msb[:, _la_off(t) : _la_off(t) + 128]
                lb = msb[:, _lb_off(t) : _lb_off(t) + 128]
                if g == 0:
                    # chunk 1's data (scalar queue) usually lands before
                    # chunk 0's (sync queue) — run the c1 pair first
                    tensor.wait_ge(s_ms, 1)
                    tensor.wait_ge(s_q0, 16)
                    tensor.wait_ge(s_m, 16)
                    nc.tensor.matmul(
                        pg[:, MCHUNK : 2 * MCHUNK],
                        la,
                        msb[:, _ra_off(1) : _ra_off(1) + MCHUNK],
                        start=True,
                        stop=False,
                    )
                    nc.tensor.matmul(
                        pg[:, MCHUNK : 2 * MCHUNK],
                        lb,
                        msb[:, _rb_off(1) : _rb_off(1) + MCHUNK],
                        start=False,
                        stop=True,
                    )
                    tensor.wait_ge(s_cd, 16)
                    nc.tensor.matmul(
                        pg[:, 0:MCHUNK],
                        la,
                        msb[:, _ra_off(0) : _ra_off(0) + MCHUNK],
                        start=True,
                        stop=False,
                    )
                    nc.tensor.matmul(
                        pg[:, 0:MCHUNK],
                        lb,
                        msb[:, _rb_off(0) : _rb_off(0) + MCHUNK],
                        start=False,
                        stop=True,
                    )
                    tensor.wait_ge(s_g, 16)
                    tensor.wait_ge(s_h2, 16)
                    # interleave the last two chunk-pairs and release after
                    # chunk 2's pair: the first exp reads chunk 3's columns
                    # >1.4us after waking, chunk 3's pair lands ~1us earlier
                    for c in (2, 3):
                        nc.tensor.matmul(
                            pg[:, c * MCHUNK : (c + 1) * MCHUNK],
                            la,
                            msb[:, _ra_off(c) : _ra_off(c) + MCHUNK],
                            start=True,
                            stop=False,
                        )
                        mm = nc.tensor.matmul(
                            pg[:, c * MCHUNK : (c + 1) * MCHUNK],
                            lb,
                            msb[:, _rb_off(c) : _rb_off(c) + MCHUNK],
                            start=False,
                            stop=True,
                        )
                        if c == 2:
                            mm.then_inc(s_pe, 1)
                    continue
                if g == 1:
                    tensor.wait_ge(s_t1, 16)
                if g == 2:
                    tensor.wait_ge(s_t2, 16)
                if g == 3:
                    tensor.wait_ge(s_t36, 16)
                if g == 7:
                    tensor.wait_ge(s_t715, 16)
                if g == 16:
                    tensor.wait_ge(s_i, 16)
                if g >= 2:
                    tensor.wait_ge(s_act, g - 1)
                for j in range(4):
                    c = 4 * h + j
                    nc.tensor.matmul(
                        pg[:, j * MCHUNK : (j + 1) * MCHUNK],
                        la,
                        msb[:, _ra_off(c) : _ra_off(c) + MCHUNK],
                        start=True,
                        stop=False,
                    )
                if g == 16:
                    tensor.wait_ge(s_j, 16)
                for j in range(4):
                    c = 4 * h + j
                    mm = nc.tensor.matmul(
                        pg[:, j * MCHUNK : (j + 1) * MCHUNK],
                        lb,
                        msb[:, _rb_off(c) : _rb_off(c) + MCHUNK],
                        start=False,
                        stop=True,
                    )
                    if j == 1:
                        # release the group two matmuls early: the ACT reads
                        # psum sequentially — chunk 2's columns are touched
                        # ~1.0us after it wakes and chunk 3's ~1.4us, while
                        # mmB j=2/j=3 (216/432ns, already dispatched in-order
                        # behind this one) land far earlier — shortens the
                        # psum-ring round trip below the ACT-busy floor
                        mm.then_inc(s_pe, 1)

    return nc


def _bf16_split3(x):
    import ml_dtypes

    bf = ml_dtypes.bfloat16
    x = x.astype(np.float32)
    p1 = x.astype(bf)
    rem = x - p1.astype(np.float32)
    p2 = rem.astype(bf)
    rem2 = rem - p2.astype(np.float32)
    p3 = rem2.astype(bf)
    return p1, p2, p3


def _bandwidth_np(X_fit):
    # mirror of reference._bandwidth (Silverman-style)
    b, n, d = X_fit.shape
    flat = np.asarray(X_fit, dtype=np.float64).reshape(-1)
    q = np.quantile(flat, 0.75) - np.quantile(flat, 0.25)
    std = np.std(np.asarray(X_fit, dtype=np.float64).reshape(b, -1), axis=1, ddof=1)
    return (0.9 * np.minimum(std, q / 1.34) / (n**0.2)).astype(np.float32)


def _host_prep(X_query, X_fit):
    import ml_dtypes

    bf = ml_dtypes.bfloat16
    X_query = np.asarray(X_query, dtype=np.float32)
    X_fit = np.asarray(X_fit, dtype=np.float32)
    bw = _bandwidth_np(X_fit)  # [B]

    in_maps = []
    for c in range(NCORES):
        b = c // SHARDS_PER_BATCH
        s = c % SHARDS_PER_BATCH
        XQ = X_query[b, s * NSHARD : (s + 1) * NSHARD]  # [2048, 32]
        XF = X_fit[b]  # [4096, 32]
        inv_bw = np.float32(1.0) / bw[b]

        # permuted queries: tile t / partition p handles query row p*NT + t.
        # 1/bw is folded into the operands so the activation runs with an
        # immediate scale of 1.0 (no per-instruction scale-AP fetch).
        XQp = XQ.reshape(128, NT, D).transpose(1, 0, 2).reshape(NSHARD, D)
        Q = np.ascontiguousarray(
            (2.0 * np.float64(inv_bw) * XQp.T.astype(np.float64)).astype(np.float32)
        )  # [32, 2048]
        q1, q2, q3 = _bf16_split3(Q)
        FT = np.ascontiguousarray(XF.T.astype(np.float32))  # [32, 4096]
        f1, f2, f3 = _bf16_split3(FT)
        sqr = (
            FT.astype(np.float64) ** 2 * np.float64(inv_bw)
        ).astype(np.float32)  # |f|^2 / bw
        s1, s2, _s3 = _bf16_split3(sqr)

        # the per-row bias (-|q|^2/bw) rides inside mmB as two bf16 lhsT
        # rows against rhs rows of ones, so the activation needs neither a
        # bias nor a scale operand (both immediates); q3 keeps 30 of 32
        # dims to make room (the dropped q3*f1 tail is ~5e-4 on the arg)
        nx2 = (XQ.reshape(128, NT, D).astype(np.float64) ** 2).sum(-1)
        br = (
            ((-nx2) * np.float64(inv_bw)).astype(np.float32).T.reshape(1, NSHARD)
        )  # col t*128+a = bias for tile t, partition a
        b1 = br.astype(bf)
        b2 = (br - b1.astype(np.float32)).astype(bf)
        ones2 = np.ones((2, M), dtype=bf)

        la = np.concatenate([q1, q1, q1], axis=0)  # [96, 2048]
        lb = np.concatenate([q2, q2, b1, b2, q3[:30]], axis=0)
        ra = np.concatenate([f1, f2, f3, s1], axis=0)  # [128, 4096]
        rb = np.concatenate([f1, f2, ones2, f1[:30], s2], axis=0)

        meta = np.empty((128, 17), dtype=np.float32)
        meta[:, 0:16] = (-nx2 * np.float64(inv_bw)).astype(np.float32)
        meta[:, 16] = inv_bw

        blob = np.zeros((128, BLOB_W), dtype=bf)
        for t in range(NT):
            lo = OFF_Q0 if t == 0 else OFF_QT + (t - 1) * 256
            blob[0:96, lo : lo + 128] = la[:, t * 128 : (t + 1) * 128]
            blob[0:96, lo + 128 : lo + 256] = lb[:, t * 128 : (t + 1) * 128]
        blob[:, OFF_CA : OFF_CA + 512] = ra[:, 0:512]
        blob[:, OFF_DA : OFF_DA + 512] = rb[:, 0:512]
        blob[:, OFF_M : OFF_M + 34] = meta.view(np.uint16).view(bf)  # raw bytes
        blob[:, OFF_CB : OFF_CB + 512] = ra[:, 512:1024]
        blob[:, OFF_DB : OFF_DB + 512] = rb[:, 512:1024]
        blob[:, OFF_G : OFF_G + 1024] = ra[:, 1024:2048]
        blob[:, OFF_H : OFF_H + 1024] = rb[:, 1024:2048]
        blob[:, OFF_I : OFF_I + 2048] = ra[:, 2048:4096]
        blob[:, OFF_J : OFF_J + 2048] = rb[:, 2048:4096]

        in_maps.append({"blob": blob})
    return in_maps


def _gather(results):
    out = np.empty((B, N), dtype=np.float32)
    for c in range(NCORES):
        b = c // SHARDS_PER_BATCH
        s = c % SHARDS_PER_BATCH
        res = np.asarray(results[c]["res"], dtype=np.float32)  # [128, 16]
        out[b, s * NSHARD : (s + 1) * NSHARD] = res.reshape(NSHARD)
    return out


def kernel(X_query, X_fit):
    from concourse.bass_utils import run_bass_kernel_spmd

    if "nc" not in _cached:
        _cached["nc"] = _build_program()
    nc = _cached["nc"]
    in_maps = _host_prep(X_query, X_fit)
    out = run_bass_kernel_spmd(nc, in_maps, list(range(NCORES)))
    return _gather(out.results)


# revision 55
# speedup vs baseline: 1.0893x; 1.0016x over previous
"""Batched KDE kernel for Trainium2 (8 NeuronCores, SPMD).

Problem: out[b, n] = sum_m exp(-||Xq[b,n] - Xf[b,m]||^2 / bw[b])
  with Silverman bandwidth bw[b] from Xf; b=4, n=m=4096, d=32.

Sharding: data-parallel over batch b (4 batches x 2 shards of query rows
= 8 cores). Each core handles n_shard=2048 query rows against the full
m=4096 fit set of its batch.

Device algorithm (per core), raw Bass with manual semaphores:
  psum[n, m] = 2*dot - nmu2 via TWO bf16 K=128 matmuls per 512-col chunk
  (bf16 streams at 1 col/cycle; f32 values are split into bf16 pieces
  x = x1+x2+x3+O(2^-24); Q = 2*Xq^T, f = Xf^T, s = f32(f^2)):
    mmA: lhsT=[q1; q1; q1; -1]   rhs=[f1; f2; f3; s1]
    mmB: lhsT=[q2; q2; q3; -1]   rhs=[f1; f2; f1; s2]
  ScalarE activation computes exp(psum/bw - nx2/bw) with a fused
  per-partition accumulate (accum_out) -> the sum over m. ACT is the
  bottleneck (~2.05us per 2048-col group); the schedule keeps its exp
  stream dense and starts it early:
    - bias (-nx2/bw) and scale (1/bw) are host-computed, riding in the
      first scalar-queue DMA (f32 bytes bitcast into the bf16 blob)
    - inputs live in ONE dram blob ordered by first-use, split across
      BOTH HWDGE queues (sync + scalar) for ~2x head bandwidth
    - -1 rows are baked into the blob (no memsets ahead of the PE)
    - PE warmup matmuls on garbage SBUF start immediately so the HAM
      clock gate is released before the first real group
    - exp output is written back IN PLACE to the psum region it reads
      (discarded data; ScalarE->PSUM is the faster port)
    - the final acc->res reduction is split in two so only half of it
      sits after the last exp group
  NOTE: engines run in relaxed ordering mode — any same-engine RAW
  (e.g. vector reduce then add on the same column) needs an explicit
  semaphore between producer and consumer.
Host does sharding/layout/packing plus the 4 scalar bandwidth values and
query norms (global quantile needs a sort; both are O(input) prep).
"""

import numpy as np

B, N, M, D = 4, 4096, 4096, 32
NCORES = 8
SHARDS_PER_BATCH = NCORES // B  # 2
NSHARD = N // SHARDS_PER_BATCH  # 2048
NT = NSHARD // 128  # 16 n-tiles per core
MCHUNK = 512  # matmul free-dim chunk (one psum bank)
ACT_FD = 2048  # activation free dim (4 psum banks)
NG = NT * (M // ACT_FD)  # 32 matmul/exp groups

# blob column offsets (bf16 cols, 64-aligned), ordered by first use and
# grouped into contiguous per-queue transfer ranges. q-piece regions hold
# only 96 data rows (the -1 rows are memset on-device); per-tile lhsT
# pairs [la_t | lb_t] are contiguous so each tile is one small transfer.
OFF_Q0 = 0  # la_t0 (128) | lb_t0 (128)
OFF_CA = 256  # ra m[0:512)
OFF_DA = 768  # rb m[0:512)
OFF_M = 1280  # meta: f32 [128,17] bitcast -> 34 bf16 cols (padded to 64)
OFF_CB = 1344  # ra m[512:1024)
OFF_DB = 1856  # rb m[512:1024)
OFF_G = 2368  # ra m[1024:2048)
OFF_H = 3392  # rb m[1024:2048)
OFF_QT = 4416  # tiles 1..15: [la_t | lb_t] x 256 cols each
OFF_I = 8256  # ra m[2048:4096) (2048)
OFF_J = 10304  # rb m[2048:4096) (2048)
BLOB_W = 12352

_cached = {}


def _la_off(t):
    if t == 0:
        return OFF_Q0
    return OFF_QT + (t - 1) * 256


def _lb_off(t):
    if t == 0:
        return OFF_Q0 + 128
    return OFF_QT + (t - 1) * 256 + 128


def _ra_off(c):  # c = m-col / 512, 0..7
    if c == 0:
        return OFF_CA
    if c == 1:
        return OFF_CB
    if c <= 3:
        return OFF_G + (c - 2) * 512
    return OFF_I + (c - 4) * 512


def _rb_off(c):
    if c == 0:
        return OFF_DA
    if c == 1:
        return OFF_DB
    if c <= 3:
        return OFF_H + (c - 2) * 512
    return OFF_J + (c - 4) * 512


def _build_program():
    import concourse.bass as bass
    import concourse.mybir as mybir
    from contextlib import ExitStack

    nc = bass.Bass()
    f32 = mybir.dt.float32
    bf16 = mybir.dt.bfloat16

    blob = nc.declare_dram_parameter("blob", [128, BLOB_W], bf16, isOutput=False)
    res = nc.declare_dram_parameter("res", [128, NT], f32, isOutput=True)

    with ExitStack() as ctx:
        msb = ctx.enter_context(nc.sbuf_tensor([128, BLOB_W], bf16))
        escr0 = ctx.enter_context(nc.sbuf_tensor([128, ACT_FD], bf16))
        escr1 = ctx.enter_context(nc.sbuf_tensor([128, ACT_FD], bf16))
        escr = [escr0, escr1]
        # slot 2t+h per group; slot NG = split-off first half of group 0
        acc = ctx.enter_context(nc.sbuf_tensor([128, NG + 1], f32))
        res_sb = ctx.enter_context(nc.sbuf_tensor([128, NT], f32))
        warmT = ctx.enter_context(nc.sbuf_tensor([128, 1], f32))
        wscr = ctx.enter_context(nc.sbuf_tensor([128, 640], bf16))
        ps0 = ctx.enter_context(nc.psum_tensor("ps0", [128, ACT_FD], f32))
        ps1 = ctx.enter_context(nc.psum_tensor("ps1", [128, ACT_FD], f32))
        ps = [ps0, ps1]

        s_q0 = ctx.enter_context(nc.semaphore("s_q0"))
        s_cd = ctx.enter_context(nc.semaphore("s_cd"))
        s_m = ctx.enter_context(nc.semaphore("s_m"))
        s_g = ctx.enter_context(nc.semaphore("s_g"))
        s_h2 = ctx.enter_context(nc.semaphore("s_h2"))
        s_t1 = ctx.enter_context(nc.semaphore("s_t1"))
        s_t2 = ctx.enter_context(nc.semaphore("s_t2"))
        s_t36 = ctx.enter_context(nc.semaphore("s_t36"))
        s_t715 = ctx.enter_context(nc.semaphore("s_t715"))
        s_i = ctx.enter_context(nc.semaphore("s_i"))
        s_j = ctx.enter_context(nc.semaphore("s_j"))
        s_ms = ctx.enter_context(nc.semaphore("s_ms"))
        s_pe = ctx.enter_context(nc.semaphore("s_pe"))
        s_act = ctx.enter_context(nc.semaphore("s_act"))
        s_v1 = ctx.enter_context(nc.semaphore("s_v1"))
        s_v2 = ctx.enter_context(nc.semaphore("s_v2"))
        sem_out = ctx.enter_context(nc.semaphore("sem_out"))
        block = ctx.enter_context(nc.Block())

        meta32 = msb[:, OFF_M : OFF_M + 34].bitcast(f32)  # [128, 17]
        # meta32[:, t] = -nx2/bw for tile t; 1/bw is folded into the
        # matmul operands, so activations use scale=1.0 (immediate)

        @block.sync
        def _(sync):
            sync.dma_start(
                msb[0:96, OFF_Q0:OFF_CA], blob[0:96, OFF_Q0:OFF_CA]
            ).then_inc(s_q0, 16)
            sync.dma_start(msb[:, OFF_CA:OFF_M], blob[:, OFF_CA:OFF_M]).then_inc(
                s_cd, 16
            )
            sync.dma_start(msb[:, OFF_G:OFF_H], blob[:, OFF_G:OFF_H]).then_inc(
                s_g, 16
            )
            sync.dma_start(
                msb[0:96, OFF_QT : OFF_QT + 256], blob[0:96, OFF_QT : OFF_QT + 256]
            ).then_inc(s_t1, 16)
            sync.dma_start(
                msb[0:96, OFF_QT + 512 : OFF_QT + 1536],
                blob[0:96, OFF_QT + 512 : OFF_QT + 1536],
            ).then_inc(s_t36, 16)
            sync.dma_start(
                msb[0:96, OFF_QT + 256 : OFF_QT + 512],
                blob[0:96, OFF_QT + 256 : OFF_QT + 512],
            ).then_inc(s_t2, 16)
            sync.dma_start(
                msb[0:96, OFF_QT + 1536 : OFF_I], blob[0:96, OFF_QT + 1536 : OFF_I]
            ).then_inc(s_t715, 16)
            sync.dma_start(msb[:, OFF_I:OFF_J], blob[:, OFF_I:OFF_J]).then_inc(
                s_i, 16
            )
            sync.dma_start(msb[:, OFF_J:BLOB_W], blob[:, OFF_J:BLOB_W]).then_inc(
                s_j, 16
            )
            sync.wait_ge(s_v1, 1)
            sync.dma_start(res[:, 0:8], res_sb[:, 0:8]).then_inc(sem_out, 16)
            sync.wait_ge(s_v2, 1)
            # no completion wait: the NEFF teardown drains the DMA queue;
            # skipping it lets the block exit overlap the HBM write receipt
            sync.dma_start(res[:, 8:16], res_sb[:, 8:16]).then_inc(sem_out, 16)

        @block.vector
        def _(vector):
            # -1 rows (96:128) of the q-piece regions, off the DMA path
            nc.vector.memset(msb[96:128, OFF_Q0:OFF_CA], -1.0)
            nc.vector.memset(msb[96:128, OFF_QT:OFF_I], -1.0).then_inc(s_ms, 1)
            # split final reduction: tiles 0-7 as soon as their h=1 groups
            # are done, tiles 8-15 after the last group
            vector.wait_ge(s_act, 1 + 23)
            nc.vector.tensor_reduce(
                res_sb[:, 0:8],
                acc[:, 0:16].rearrange("p (t h) -> p t h", h=2),
                axis=mybir.AxisListType.X,
                op=mybir.AluOpType.add,
            ).then_inc(s_v1, 1)
            vector.wait_ge(s_act, 1 + 31)
            nc.vector.tensor_reduce(
                res_sb[:, 8:16],
                acc[:, 16:32].rearrange("p (t h) -> p t h", h=2),
                axis=mybir.AxisListType.X,
                op=mybir.AluOpType.add,
            ).then_inc(s_v2, 1)

        @block.scalar
        def _(scalar):
            # second HWDGE queue: scalar-issued DMAs interleave with the
            # sync queue on the shared port; order both queues by first use
            scalar.dma_start(msb[:, OFF_M:OFF_G], blob[:, OFF_M:OFF_G]).then_inc(
                s_m, 16
            )
            scalar.dma_start(msb[:, OFF_H:OFF_QT], blob[:, OFF_H:OFF_QT]).then_inc(
                s_h2, 16
            )
            # fire the exp table-set load; operands are garbage (meta not
            # yet DMA'd) but the output is discarded
            nc.scalar.activation(
                warmT[:],
                warmT[:],
                mybir.ActivationFunctionType.Exp,
                bias=meta32[:, 0:1],
            )
            scalar.wait_ge(s_m, 16)
            # uniform 2048-col groups; s_pe = g+1 when group g's psum is
            # filled, s_act = g+1 when its exp+accumulate is done
            for g in range(NG):
                t = g % NT
                slot = 2 * t + (g // NT)
                scalar.wait_ge(s_pe, g + 1)
                # direct InstActivation: bias/scale as immediates (the
                # bias is folded into mmB, the scale into the operands) —
                # saves two per-instruction operand-AP fetches (~180ns)
                nc.scalar.add_instruction(
                    mybir.InstActivation(
                        name=nc.get_next_instruction_name(),
                        func=mybir.ActivationFunctionType.Exp,
                        ins=[
                            nc.scalar.lower_ap(ps[g % 2][:]),
                            mybir.ImmediateValue(
                                dtype=mybir.dt.float32, value=0.0
                            ),
                            mybir.ImmediateValue(
                                dtype=mybir.dt.float32, value=1.0
                            ),
                            mybir.ImmediateValue(
                                dtype=mybir.dt.float32, value=0.0
                            ),
                        ],
                        outs=[
                            nc.scalar.lower_ap(escr[g % 2][:]),
                            nc.scalar.lower_ap(acc[:, slot : slot + 1]),
                        ],
                    )
                ).then_inc(s_act, 1)

        @block.tensor
        def _(tensor):
            # warm the PE clock (HAM) with dummy matmuls on garbage SBUF so
            # group 0 runs at 2.4 GHz; ps0 is overwritten by group 0
            for _w in range(10):
                nc.tensor.matmul(
                    ps0[:, 0:MCHUNK],
                    wscr[:, 0:128],
                    wscr[:, 128:640],
                    start=True,
                    stop=True,
                )
            for g in range(NG):
                t = g % NT
                h = g // NT
                pg = ps[g % 2]
                la = msb[:, _la_off(t) : _la_off(t) + 128]
                lb = msb[:, _lb_off(t) : _lb_off(t) + 128]
                if g == 0:
                    # chunk 1's data (scalar queue) usually lands before
                    # chunk 0's (sync queue) — run the c1 pair first
                    tensor.wait_ge(s_ms, 1)
                    tensor.wait_ge(s_q0, 16)
                    tensor.wait_ge(s_m, 16)
                    nc.tensor.matmul(
                        pg[:, MCHUNK : 2 * MCHUNK],
                        la,
                        msb[:, _ra_off(1) : _ra_off(1) + MCHUNK],
                        start=True,
                        stop=False,
                    )
                    nc.tensor.matmul(
                        pg[:, MCHUNK : 2 * MCHUNK],
                        lb,
                        msb[:, _rb_off(1) : _rb_off(1) + MCHUNK],
                        start=False,
                        stop=True,
                    )
                    tensor.wait_ge(s_cd, 16)
                    nc.tensor.matmul(
                        pg[:, 0:MCHUNK],
                        la,
                        msb[:, _ra_off(0) : _ra_off(0) + MCHUNK],
                        start=True,
                        stop=False,
                    )
                    nc.tensor.matmul(
                        pg[:, 0:MCHUNK],
                        lb,
                        msb[:, _rb_off(0) : _rb_off(0) + MCHUNK],
                        start=False,
                        stop=True,
                    )
                    tensor.wait_ge(s_g, 16)
                    tensor.wait_ge(s_h2, 16)
                    # interleave the last two chunk-pairs and release after
                    # chunk 2's pair: the first exp reads chunk 3's columns
                    # >1.4us after waking, chunk 3's pair lands ~1us earlier
                    for c in (2, 3):
                        mm = nc.tensor.matmul(
                            pg[:, c * MCHUNK : (c + 1) * MCHUNK],
                            la,
                            msb[:, _ra_off(c) : _ra_off(c) + MCHUNK],
                            start=True,
                            stop=False,
                        )
                        if c == 2:
                            # both DMA waits resolved above; everything
                            # after this point is engine-timed and lands
                            # well ahead of the exp's sequential read
                            mm.then_inc(s_pe, 1)
                        nc.tensor.matmul(
                            pg[:, c * MCHUNK : (c + 1) * MCHUNK],
                            lb,
                            msb[:, _rb_off(c) : _rb_off(c) + MCHUNK],
                            start=False,
                            stop=True,
                        )
                    continue
                if g == 1:
                    tensor.wait_ge(s_t1, 16)
                if g == 2:
                    tensor.wait_ge(s_t2, 16)
                if g == 3:
                    tensor.wait_ge(s_t36, 16)
                if g == 7:
                    tensor.wait_ge(s_t715, 16)
                if g == 16:
                    tensor.wait_ge(s_i, 16)
                if g >= 2:
                    tensor.wait_ge(s_act, g - 1)
                for j in range(4):
                    c = 4 * h + j
                    nc.tensor.matmul(
                        pg[:, j * MCHUNK : (j + 1) * MCHUNK],
                        la,
                        msb[:, _ra_off(c) : _ra_off(c) + MCHUNK],
                        start=True,
                        stop=False,
                    )
                if g == 16:
                    tensor.wait_ge(s_j, 16)
                for j in range(4):
                    c = 4 * h + j
                    mm = nc.tensor.matmul(
                        pg[:, j * MCHUNK : (j + 1) * MCHUNK],
                        lb,
                        msb[:, _rb_off(c) : _rb_off(c) + MCHUNK],
                        start=False,
                        stop=True,
                    )
                    if j == 1:
                        # release the group two matmuls early: the ACT reads
                        # psum sequentially — chunk 2's columns are touched
                        # ~1.0us after it wakes and chunk 3's ~1.4us, while
                        # mmB j=2/j=3 (216/432ns, already dispatched in-order
                        # behind this one) land far earlier — shortens the
                        # psum-ring round trip below the ACT-busy floor
                        mm.then_inc(s_pe, 1)

    return nc


def _bf16_split3(x):
    import ml_dtypes

    bf = ml_dtypes.bfloat16
    x = x.astype(np.float32)
    p1 = x.astype(bf)
    rem = x - p1.astype(np.float32)
    p2 = rem.astype(bf)
    rem2 = rem - p2.astype(np.float32)
    p3 = rem2.astype(bf)
    return p1, p2, p3


def _bandwidth_np(X_fit):
    # mirror of reference._bandwidth (Silverman-style)
    b, n, d = X_fit.shape
    flat = np.asarray(X_fit, dtype=np.float64).reshape(-1)
    q = np.quantile(flat, 0.75) - np.quantile(flat, 0.25)
    std = np.std(np.asarray(X_fit, dtype=np.float64).reshape(b, -1), axis=1, ddof=1)
    return (0.9 * np.minimum(std, q / 1.34) / (n**0.2)).astype(np.float32)


def _host_prep(X_query, X_fit):
    import ml_dtypes

    bf = ml_dtypes.bfloat16
    X_query = np.asarray(X_query, dtype=np.float32)
    X_fit = np.asarray(X_fit, dtype=np.float32)
    bw = _bandwidth_np(X_fit)  # [B]

    in_maps = []
    for c in range(NCORES):
        b = c // SHARDS_PER_BATCH
        s = c % SHARDS_PER_BATCH
        XQ = X_query[b, s * NSHARD : (s + 1) * NSHARD]  # [2048, 32]
        XF = X_fit[b]  # [4096, 32]
        inv_bw = np.float32(1.0) / bw[b]

        # permuted queries: tile t / partition p handles query row p*NT + t.
        # 1/bw is folded into the operands so the activation runs with an
        # immediate scale of 1.0 (no per-instruction scale-AP fetch).
        XQp = XQ.reshape(128, NT, D).transpose(1, 0, 2).reshape(NSHARD, D)
        Q = np.ascontiguousarray(
            (2.0 * np.float64(inv_bw) * XQp.T.astype(np.float64)).astype(np.float32)
        )  # [32, 2048]
        q1, q2, q3 = _bf16_split3(Q)
        FT = np.ascontiguousarray(XF.T.astype(np.float32))  # [32, 4096]
        f1, f2, f3 = _bf16_split3(FT)
        sqr = (
            FT.astype(np.float64) ** 2 * np.float64(inv_bw)
        ).astype(np.float32)  # |f|^2 / bw
        s1, s2, _s3 = _bf16_split3(sqr)

        # the per-row bias (-|q|^2/bw) rides inside mmB as two bf16 lhsT
        # rows against rhs rows of ones, so the activation needs neither a
        # bias nor a scale operand (both immediates); q3 keeps 30 of 32
        # dims to make room (the dropped q3*f1 tail is ~5e-4 on the arg)
        nx2 = (XQ.reshape(128, NT, D).astype(np.float64) ** 2).sum(-1)
        br = (
            ((-nx2) * np.float64(inv_bw)).astype(np.float32).T.reshape(1, NSHARD)
        )  # col t*128+a = bias for tile t, partition a
        b1 = br.astype(bf)
        b2 = (br - b1.astype(np.float32)).astype(bf)
        ones2 = np.ones((2, M), dtype=bf)

        la = np.concatenate([q1, q1, q1], axis=0)  # [96, 2048]
        lb = np.concatenate([q2, q2, b1, b2, q3[:30]], axis=0)
        ra = np.concatenate([f1, f2, f3, s1], axis=0)  # [128, 4096]
        rb = np.concatenate([f1, f2, ones2, f1[:30], s2], axis=0)

        meta = np.empty((128, 17), dtype=np.float32)
        meta[:, 0:16] = (-nx2 * np.float64(inv_bw)).astype(np.float32)
        meta[:, 16] = inv_bw

        blob = np.zeros((128, BLOB_W), dtype=bf)
        for t in range(NT):
            lo = OFF_Q0 if t == 0 else OFF_QT + (t - 1) * 256
            blob[0:96, lo : lo + 128] = la[:, t * 128 : (t + 1) * 128]
            blob[0:96, lo + 128 : lo + 256] = lb[:, t * 128 : (t + 1) * 128]
        blob[:, OFF_CA : OFF_CA + 512] = ra[:, 0:512]
        blob[:, OFF_DA : OFF_DA + 512] = rb[:, 0:512]
        blob[:, OFF_M : OFF_M + 34] = meta.view(np.uint16).view(bf)  # raw bytes
        blob[:, OFF_CB : OFF_CB + 512] = ra[:, 512:1024]
        blob[:, OFF_DB : OFF_DB + 512] = rb[:, 512:1024]
        blob[:, OFF_G : OFF_G + 1024] = ra[:, 1024:2048]
        blob[:, OFF_H : OFF_H + 1024] = rb[:, 1024:2048]
        blob[:, OFF_I : OFF_I + 2048] = ra[:, 2048:4096]
        blob[:, OFF_J : OFF_J + 2048] = rb[:, 2048:4096]

        in_maps.append({"blob": blob})
    return in_maps


def _gather(results):
    out = np.empty((B, N), dtype=np.float32)
    for c in range(NCORES):
        b = c // SHARDS_PER_BATCH
        s = c % SHARDS_PER_BATCH
        res = np.asarray(results[c]["res"], dtype=np.float32)  # [128, 16]
        out[b, s * NSHARD : (s + 1) * NSHARD] = res.reshape(NSHARD)
    return out


def kernel(X_query, X_fit):
    from concourse.bass_utils import run_bass_kernel_spmd

    if "nc" not in _cached:
        _cached["nc"] = _build_program()
    nc = _cached["nc"]
    in_maps = _host_prep(X_query, X_fit)
    out = run_bass_kernel_spmd(nc, in_maps, list(range(NCORES)))
    return _gather(out.results)
